# revision 24
# baseline (speedup 1.0000x reference)
"""Trainium2 Bass kernel for nn_Discriminator (2-layer LSTM, B=512 T=100 H=1024).

Strategy: data-parallel over batch across 8 cores (B=64 per core), with
both LSTM layers FUSED into one software-pipelined loop: macro-step t
computes layer-1 cell for time t and layer-2 cell for time t-2 (lag 2).
Gate preactivations accumulate in per-layer [64, 1024] PSUM tiles
(DoubleRow matmuls can only write PSUM partition 0), but the
activations write into ONE partition-stacked SBUF tile (L1 -> rows
0:64, L2 -> rows 64:128), so all downstream elementwise work (i*g, c
update, tanh, h) and the h^T transposes run once on [128, .] tiles for
both layers, and the PE always has >20us of mostly-independent matmul
work per step to hide the serial activation chain.

  - Recurrence products (h1@W_hh1^T, h2@W_hh2^T) are fp8e4m3 DoubleRow
    matmuls reading a shared fp8 h^T ring (cols 0:64 = h1, 64:128 =
    h2).  Weights are scaled x32 and h x16 before the fp8 cast
    (U(+-0.031) weights / small h are subnormal in e4m3 otherwise ->
    10-20% quantization error) and descaled for free via the
    activation `scale` operand.  Measured on HW: DR gives no
    per-instruction speedup (cost = N_out cycles regardless), but
    doubles K per instruction, which is what makes M=64 recurrences
    run at the same MAC rate as M=128 bf16 GEMMs.
  - The layer-2 input product h1@W_ih2^T MUST be bf16 on both operands
    (any fp8 operand pushes total rel err past the 2e-2 gate).  It is
    batched 2 timesteps at a time as an M=128 bf16 GEMM from a bf16
    h1^T ring (lag-2 makes both steps' h1 available), evicted
    PSUM->SBUF with the layer-2 bias added, then DVE-preset into the
    gate PSUM before the W_hh2 DoubleRow matmuls accumulate onto it
    (start=False onto DVE-written PSUM is legal).
  - Layer-1's input projection collapses through the encoder:
    W_comb = W_ih1 @ enc_W^T, and the per-step preload x_t @ W_comb^T
    rides a K=35 matmul whose lhsT carries [x_t ; ones]: the combined
    layer-1 bias lands with the projection.
  - Weight builds evict transposed k-pairs ([128, 2, 128]) in single
    DVE/Act ops -- evictions, not DMA, gate the prologue.
  - No DRAM scratch at all; HBM traffic is weights + x (~67MB/core,
    vs 199MB for the phase-separated baseline).  4.13ms -> 3.54ms.
"""

import numpy as np

import concourse.bass as bass
import concourse.tile as tile
import concourse.mybir as mybir
from concourse import bacc
from concourse.bass_utils import run_bass_kernel_spmd
from concourse.masks import make_identity

F32 = mybir.dt.float32
F32R = mybir.dt.float32r
BF16 = mybir.dt.bfloat16
FP8 = mybir.dt.float8e4
AF = mybir.ActivationFunctionType
DR = mybir.MatmulPerfMode.DoubleRow

N_CORES = 8
B, IN, H = 512, 34, 1024
G = 4 * H                 # 4096
BPC = B // N_CORES        # 64 batch rows per core
P = 128
KC = H // P               # 8 contraction chunks
KPF = KC // 2             # 4 fp8 k-pairs
NSLOT = 4                 # h^T ring depth
HDT = BF16
HF = 512                  # half of H for the split serial tail

WSCALE = 32.0             # fp8 weight pre-scale (exact power of 2)
HSCALE = 16.0             # fp8 h-ring pre-scale (fixes subnormal h)
INV_WSCALE = 1.0 / (WSCALE * HSCALE)


def _build_weight_T8(nc, w_dram, w_T8, identr, wrow, wtr_ps):
    """Transpose w_dram [G, H] into resident fp8 SBUF tile
    w_T8 [128, KPF, 2, G] (k-pair packed for DoubleRow), scaled by
    WSCALE in the PSUM->SBUF eviction (the PE transpose datapath
    ignores the identity operand's values, so scaling must not ride
    the transpose itself)."""
    n_row_tiles = w_dram.shape[0] // P  # 32
    for r in range(n_row_tiles):
        wt = wrow.tile([P, H], F32R, tag="wrow")
        nc.sync.dma_start(wt[:], w_dram[r * P:(r + 1) * P, :].bitcast(F32R))
        for kp in range(KPF):
            # two chunk transposes into one tile, ONE k-pair eviction
            # (evictions gate the build; halve their count)
            pt = wtr_ps.tile([P, 2, P], F32R, tag="wtr")
            for e in range(2):
                c = 2 * kp + e
                nc.tensor.transpose(pt[:, e, :], wt[:, c * P:(c + 1) * P],
                                    identr[:])
            dst = w_T8[:, kp, :, r * P:(r + 1) * P]
            # gpsimd cannot touch PSUM; alternate vector / scalar(Copy)
            if kp % 2 == 0:
                nc.vector.tensor_scalar_mul(dst, pt[:], WSCALE)
            else:
                nc.scalar.activation(dst, pt[:], AF.Copy, scale=WSCALE)


def build(T=100):
    nc = bacc.Bacc("TRN2", target_bir_lowering=False, debug=False,
                   num_devices=N_CORES)

    x = nc.dram_tensor("x", [BPC, T, IN], F32, kind="ExternalInput").ap()
    enc_W = nc.dram_tensor("enc_W", [H, IN], F32, kind="ExternalInput").ap()
    enc_b = nc.dram_tensor("enc_b", [H], F32, kind="ExternalInput").ap()
    W_ih1 = nc.dram_tensor("W_ih1", [G, H], F32, kind="ExternalInput").ap()
    W_hh1 = nc.dram_tensor("W_hh1", [G, H], F32, kind="ExternalInput").ap()
    b_ih1 = nc.dram_tensor("b_ih1", [G], F32, kind="ExternalInput").ap()
    b_hh1 = nc.dram_tensor("b_hh1", [G], F32, kind="ExternalInput").ap()
    W_ih2 = nc.dram_tensor("W_ih2", [G, H], F32, kind="ExternalInput").ap()
    W_hh2 = nc.dram_tensor("W_hh2", [G, H], F32, kind="ExternalInput").ap()
    b_ih2 = nc.dram_tensor("b_ih2", [G], F32, kind="ExternalInput").ap()
    b_hh2 = nc.dram_tensor("b_hh2", [G], F32, kind="ExternalInput").ap()
    dec_W = nc.dram_tensor("dec_W", [1, H], F32, kind="ExternalInput").ap()
    dec_b = nc.dram_tensor("dec_b", [1], F32, kind="ExternalInput").ap()
    out = nc.dram_tensor("out", [BPC, 1], F32, kind="ExternalOutput").ap()

    with tile.TileContext(nc) as tc:
        with tc.tile_pool(name="persist", bufs=1) as persist, \
             tc.tile_pool(name="state", bufs=1) as state, \
             tc.tile_pool(name="misc", bufs=1) as misc:

            ident = persist.tile([P, P], F32, tag="ident")
            make_identity(nc, ident[:])
            identr = persist.tile([P, P], F32R, tag="identr")
            nc.vector.tensor_copy(identr[:], ident[:])
            identh = persist.tile([P, P], HDT, tag="identh")
            nc.vector.tensor_copy(identh[:], ident[:])
            zb = persist.tile([P, 1], F32, tag="zero_bias")
            nc.gpsimd.memset(zb[:], 0.0)

            # layer-1 input-side operands, pre-scaled by WSCALE so the
            # PSUM accumulation matches the fp8 DR product scale:
            #   W_combT [35, G]: rows 0:34 = 32*(W_ih1@enc_W)^T,
            #     row 34 = 32*(enc_b@W_ih1^T + b_ih1 + b_hh1)
            #   xTa [35, T, 64]: per step t: [x_t^T ; ones]
            W_combT = persist.tile([IN + 1, G], BF16, tag="W_combT")
            xTa = persist.tile([IN + 1, T, BPC], BF16, tag="xTa")
            ones1 = persist.tile([1, P], F32R, tag="ones1")
            nc.gpsimd.memset(ones1[:].bitcast(F32), 1.0)
            # layer-2 bias broadcast [128, G], pre-scaled by WSCALE*HSCALE
            bias128_2 = persist.tile([P, G], BF16, tag="bias128_2")

            # ============ Phase E: xTa ============
            with nc.named_scope("phaseE"):
                with tc.tile_pool(name="e_sb", bufs=1) as e_sb, \
                     tc.tile_pool(name="e_ps", bufs=3, space="PSUM") as e_ps:
                    onesrow = e_sb.tile([1, T, BPC], BF16, tag="onesrow")
                    nc.gpsimd.memset(onesrow[:], 1.0)
                    nc.sync.dma_start(xTa[IN:IN + 1, :, :], onesrow[:])
                    # one bulk contiguous DMA of all of x (13.6KB/partition)
                    xall = e_sb.tile([BPC, T, IN], F32R, tag="xall")
                    nc.sync.dma_start(xall[:], x.bitcast(F32R))
                    for m in range(T // 2):
                        pt = e_ps.tile([IN, P], F32R, tag="xtr")
                        for e in range(2):
                            nc.tensor.transpose(
                                pt[:, e * BPC:(e + 1) * BPC],
                                xall[:, 2 * m + e, :], identr[0:BPC, 0:BPC])
                        nc.vector.tensor_copy(xTa[0:IN, 2 * m, :], pt[:, 0:BPC])
                        nc.scalar.activation(xTa[0:IN, 2 * m + 1, :], pt[:, BPC:P], AF.Copy)

            # ============ W_combT build (incremental, scaled) ============
            with nc.named_scope("build_Wcomb"):
                with tc.tile_pool(name="wc_sb", bufs=1) as wc_sb, \
                     tc.tile_pool(name="wc_row", bufs=6) as wc_row, \
                     tc.tile_pool(name="wc_st", bufs=2) as wc_st, \
                     tc.tile_pool(name="wc_ps", bufs=2, space="PSUM") as wc_ps, \
                     tc.tile_pool(name="wc_ps2", bufs=1, space="PSUM") as wc_ps2:
                    encwb = wc_sb.tile([P, KC, IN], F32R, tag="encwb")
                    nc.sync.dma_start(
                        encwb[:],
                        enc_W.rearrange("(c p) f -> p c f", p=P).bitcast(F32R))
                    encb_k = wc_sb.tile([P, KC], F32R, tag="encb_k")
                    nc.sync.dma_start(
                        encb_k[:],
                        enc_b.rearrange("(c p) -> p c", p=P).bitcast(F32R))
                    # WSCALE rides the encoder-side operands of the
                    # W_comb contraction (transposes don't scale)
                    nc.vector.tensor_scalar_mul(encwb[:], encwb[:], WSCALE * HSCALE)
                    nc.vector.tensor_scalar_mul(encb_k[:], encb_k[:], WSCALE * HSCALE)
                    brow1 = wc_sb.tile([1, G], BF16, tag="brow1")
                    bsum1 = wc_sb.tile([1, G], F32, tag="bsum1")
                    # pre-scaled bias sums; transient pool so the [1, G]
                    # f32 scratch frees before the weight staging runs
                    with tc.tile_pool(name="wc_tmp", bufs=1) as wc_tmp:
                        tA = wc_tmp.tile([1, G], F32, tag="tA")
                        nc.sync.dma_start(tA[:], b_ih1[None, :])
                        tB = wc_tmp.tile([1, G], F32, tag="tB")
                        nc.sync.dma_start(tB[:], b_hh1[None, :])
                        nc.vector.tensor_add(bsum1[:], tA[:], tB[:])
                        nc.gpsimd.tensor_scalar_mul(bsum1[:], bsum1[:], WSCALE * HSCALE)
                        tA = wc_tmp.tile([1, G], F32, tag="tA")
                        nc.sync.dma_start(tA[:], b_ih2[None, :])
                        tB = wc_tmp.tile([1, G], F32, tag="tB")
                        nc.sync.dma_start(tB[:], b_hh2[None, :])
                        nc.vector.tensor_add(tA[:], tA[:], tB[:])
                        nc.gpsimd.tensor_scalar_mul(tA[:], tA[:],
                                                    WSCALE * HSCALE)
                        brow2 = wc_tmp.tile([1, G], F32R, tag="brow2")
                        nc.vector.tensor_copy(brow2[:], tA[:])
                        for n in range(8):
                            slb = slice(n * 512, (n + 1) * 512)
                            pbb = wc_ps.tile([P, 512], F32, tag="pbb")
                            nc.tensor.matmul(pbb[:], ones1[:], brow2[:, slb],
                                             start=True, stop=True)
                            nc.vector.tensor_copy(bias128_2[:, slb], pbb[:])
                    # groups of 4 row-chunks = 512 G columns
                    for grp in range(G // 512):
                        wstage = wc_st.tile([P, KC, 512], F32R, tag="wstage")
                        for rr in range(4):
                            r = grp * 4 + rr
                            wt = wc_row.tile([P, H], F32R, tag="wcrow")
                            nc.sync.dma_start(
                                wt[:], W_ih1[r * P:(r + 1) * P, :].bitcast(F32R))
                            for kp in range(KPF):
                                ptr = wc_ps.tile([P, 2, P], F32R, tag="wctr")
                                for e in range(2):
                                    c = 2 * kp + e
                                    nc.tensor.transpose(
                                        ptr[:, e, :], wt[:, c * P:(c + 1) * P],
                                        identr[:])
                                dstw = wstage[:, 2 * kp:2 * kp + 2,
                                              rr * P:(rr + 1) * P]
                                if kp % 2 == 0:
                                    nc.vector.tensor_copy(dstw, ptr[:])
                                else:
                                    nc.scalar.activation(dstw, ptr[:], AF.Copy)
                        pb = wc_ps2.tile([IN, 512], F32, tag="wcpb")
                        pbias = wc_ps2.tile([1, 512], F32, tag="wcpbias")
                        for k in range(KC):
                            nc.tensor.matmul(pb[:], encwb[:, k, :],
                                             wstage[:, k, :],
                                             start=(k == 0), stop=(k == KC - 1))
                        for k in range(KC):
                            nc.tensor.matmul(pbias[:], encb_k[:, k:k + 1],
                                             wstage[:, k, :],
                                             start=(k == 0), stop=(k == KC - 1))
                        sl = slice(grp * 512, (grp + 1) * 512)
                        nc.vector.tensor_copy(W_combT[0:IN, sl], pb[:])
                        nc.vector.tensor_add(brow1[:, sl], pbias[:], bsum1[:, sl])
                    # bias row rides as contraction row 34 (DMA can hit
                    # the unaligned partition offset)
                    nc.sync.dma_start(W_combT[IN:IN + 1, :], brow1[:])

            # ============ fp8 weight builds (all resident) ============
            with tc.tile_pool(name="wpool", bufs=1) as wpool:
                w1 = wpool.tile([P, KPF, 2, G], FP8, tag="Whh1")
                w2h = wpool.tile([P, KPF, 2, G], FP8, tag="Whh2")
                w2b = wpool.tile([P, KC, G], BF16, tag="Wih2b")
                with nc.named_scope("build_W8"):
                    with tc.tile_pool(name="wrow1", bufs=6) as wrow, \
                         tc.tile_pool(name="wtr_ps1", bufs=3, space="PSUM") as wtr_ps:
                        _build_weight_T8(nc, W_hh1, w1, identr, wrow, wtr_ps)
                        _build_weight_T8(nc, W_hh2, w2h, identr, wrow, wtr_ps)
                        # W_ih2^T in bf16 (x512) for the batched A2 GEMM
                        for r in range(G // P):
                            wt = wrow.tile([P, H], F32R, tag="wrow")
                            nc.sync.dma_start(
                                wt[:], W_ih2[r * P:(r + 1) * P, :].bitcast(F32R))
                            for kp in range(KPF):
                                pt = wtr_ps.tile([P, 2, P], F32R, tag="wtr")
                                for e in range(2):
                                    c = 2 * kp + e
                                    nc.tensor.transpose(
                                        pt[:, e, :], wt[:, c * P:(c + 1) * P],
                                        identr[:])
                                dst = w2b[:, 2 * kp:2 * kp + 2,
                                          r * P:(r + 1) * P]
                                if kp % 2 == 0:
                                    nc.vector.tensor_scalar_mul(dst, pt[:],
                                                                WSCALE * HSCALE)
                                else:
                                    nc.scalar.activation(dst, pt[:], AF.Copy,
                                                         scale=WSCALE * HSCALE)

                # persistent state
                hT8 = state.tile([P, KPF, 2, NSLOT, P], FP8, tag="hT8_ring")
                hTb = state.tile([P, KC, NSLOT, BPC], HDT, tag="hTb_ring")
                c_st = state.tile([P, H], F32, tag="c_stack")
                nc.gpsimd.memset(hT8[:].bitcast(mybir.dt.uint8), 0.0)
                nc.gpsimd.memset(hTb[:].bitcast(mybir.dt.uint16), 0.0)
                nc.gpsimd.memset(c_st[:], 0.0)

                # decode operands
                decWT_f = misc.tile([P, KC], F32, tag="decWT_f")
                nc.sync.dma_start(decWT_f[:], dec_W.rearrange("o (c p) -> p (c o)", p=P))
                decWT = misc.tile([P, KC], HDT, tag="decWT")
                nc.vector.tensor_copy(decWT[:], decWT_f[:])
                decb_f = misc.tile([1, 1], F32, tag="decb_f")
                nc.sync.dma_start(decb_f[:], dec_b[None, :])
                decb_sb = misc.tile([1, 1], HDT, tag="decb")
                nc.vector.tensor_copy(decb_sb[:], decb_f[:])
                ones_f = misc.tile([1, BPC], F32, tag="ones_f")
                nc.gpsimd.memset(ones_f[:], 1.0)
                ones_bpc = misc.tile([1, BPC], HDT, tag="ones_bpc")
                nc.vector.tensor_copy(ones_bpc[:], ones_f[:])
                hT_last = misc.tile([P, KC, BPC], HDT, tag="hT_last")

                # ============ fused recurrence loop ============
                with nc.named_scope("loop"):
                    with tc.tile_pool(name="l_g", bufs=4) as gact, \
                         tc.tile_pool(name="l_a2", bufs=2) as a2pool, \
                         tc.tile_pool(name="l_h", bufs=2) as hpool, \
                         tc.tile_pool(name="l_pg", bufs=3, space="PSUM") as psum_g, \
                         tc.tile_pool(name="l_ptr", bufs=2, space="PSUM") as psum_tr:
                        pg_next = {}
                        a2_cur = None
                        for t in range(T + 2):
                            do_l1 = t < T
                            do_l2 = t >= 2
                            r0 = 0 if do_l1 else BPC
                            r1 = P if do_l2 else BPC
                            s_r = (t - 1) % NSLOT
                            s_w = t % NSLOT

                            # batched bf16 A2 block for L2-times (t-2, t-1):
                            # a2 = [h1_{t-2}; h1_{t-1}] @ (512*W_ih2)^T + 512*b2
                            if do_l2 and t % 2 == 0:
                                a2_cur = a2pool.tile([P, G], HDT, tag="a2sb")
                                s0 = (t - 2) % NSLOT
                                for chn in range(8):
                                    cs = slice(chn * 512, (chn + 1) * 512)
                                    pa = psum_tr.tile([P, 512], F32, tag="htr",
                                                      name="pa")
                                    for k in range(KC):
                                        nc.tensor.matmul(
                                            pa[:], hTb[:, k, s0:s0 + 2, :],
                                            w2b[:, k, cs],
                                            start=(k == 0), stop=(k == KC - 1),
                                            skip_group_check=True)
                                    nc.vector.tensor_add(a2_cur[:, cs], pa[:],
                                                         bias128_2[:, cs])
                            rh = BPC * (t % 2)  # a2 row-half for L2-time t-2

                            def mk_pgA(g_idx):
                                pgA = psum_g.tile([BPC, H], F32, tag="pg",
                                                  name=f"pgA{g_idx}")
                                for n2 in range(2):
                                    n = g_idx * 2 + n2
                                    nc.tensor.matmul(
                                        pgA[:, n2 * 512:(n2 + 1) * 512],
                                        xTa[:, t, :],
                                        W_combT[:, n * 512:(n + 1) * 512],
                                        start=True, stop=False,
                                        skip_group_check=True)
                                return pgA

                            def mm_l1(g_idx, pgA):
                                for n2 in range(2):
                                    n = g_idx * 2 + n2
                                    ch = slice(n2 * 512, (n2 + 1) * 512)
                                    wch = slice(n * 512, (n + 1) * 512)
                                    for kp in range(KPF):
                                        nc.tensor.matmul(
                                            pgA[:, ch],
                                            hT8[:, kp, :, s_r, 0:BPC],
                                            w1[:, kp, :, wch],
                                            start=False, stop=(kp == KPF - 1),
                                            perf_mode=DR,
                                            skip_group_check=True)

                            def mm_l2(g_idx):
                                pgB = psum_g.tile([BPC, H], F32, tag="pg",
                                                  name=f"pgB{g_idx}")
                                # a2 (+bias) preset via DVE, then accumulate
                                nc.vector.tensor_copy(
                                    pgB[:],
                                    a2_cur[rh:rh + BPC,
                                           g_idx * H:(g_idx + 1) * H])
                                for n2 in range(2):
                                    n = g_idx * 2 + n2
                                    ch = slice(n2 * 512, (n2 + 1) * 512)
                                    wch = slice(n * 512, (n + 1) * 512)
                                    for kp in range(KPF):
                                        nc.tensor.matmul(
                                            pgB[:, ch],
                                            hT8[:, kp, :, s_r, BPC:P],
                                            w2h[:, kp, :, wch],
                                            start=False, stop=(kp == KPF - 1),
                                            perf_mode=DR,
                                            skip_group_check=True)
                                return pgB

                            acts = {}

                            def do_gate(g_idx, func, name):
                                pgA = pg_next.pop(g_idx, None)
                                if do_l1:
                                    if pgA is None:
                                        pgA = mk_pgA(g_idx)
                                    mm_l1(g_idx, pgA)
                                pgB = mm_l2(g_idx) if do_l2 else None
                                at = gact.tile([P, H], HDT, tag="gact", name=name)
                                if do_l1:
                                    nc.scalar.activation(at[0:BPC], pgA[:], func,
                                                         bias=zb[0:BPC],
                                                         scale=INV_WSCALE)
                                if do_l2:
                                    nc.scalar.activation(at[BPC:P], pgB[:], func,
                                                         bias=zb[BPC:P],
                                                         scale=INV_WSCALE)
                                acts[g_idx] = at
                                return pgA, pgB

                            do_gate(0, AF.Sigmoid, "act_i")
                            do_gate(2, AF.Tanh, "act_g")
                            tmp = gact.tile([P, H], HDT, tag="gact", name="tmp")
                            nc.vector.tensor_mul(tmp[r0:r1], acts[0][r0:r1],
                                                 acts[2][r0:r1])

                            # gate f, then c update + tanh(c), in halves
                            pgA_f = pg_next.pop(1, None)
                            if do_l1:
                                if pgA_f is None:
                                    pgA_f = mk_pgA(1)
                                mm_l1(1, pgA_f)
                            pgB_f = mm_l2(1) if do_l2 else None
                            act_f = gact.tile([P, H], HDT, tag="gact", name="act_f")
                            tanh_c = gact.tile([P, H], HDT, tag="gact", name="tanh_c")
                            for hh in (1, 0):
                                sl = slice(hh * HF, (hh + 1) * HF)
                                if do_l1:
                                    nc.scalar.activation(act_f[0:BPC, sl],
                                                         pgA_f[:, sl], AF.Sigmoid,
                                                         bias=zb[0:BPC],
                                                         scale=INV_WSCALE)
                                if do_l2:
                                    nc.scalar.activation(act_f[BPC:P, sl],
                                                         pgB_f[:, sl], AF.Sigmoid,
                                                         bias=zb[BPC:P],
                                                         scale=INV_WSCALE)
                                nc.vector.tensor_mul(c_st[r0:r1, sl],
                                                     c_st[r0:r1, sl],
                                                     act_f[r0:r1, sl])
                                nc.vector.tensor_add(c_st[r0:r1, sl],
                                                     c_st[r0:r1, sl],
                                                     tmp[r0:r1, sl])
                                nc.scalar.activation(tanh_c[r0:r1, sl],
                                                     c_st[r0:r1, sl], AF.Tanh,
                                                     bias=zb[r0:r1])

                            # gate o + h, in halves
                            pgA_o = pg_next.pop(3, None)
                            if do_l1:
                                if pgA_o is None:
                                    pgA_o = mk_pgA(3)
                                mm_l1(3, pgA_o)
                            pgB_o = mm_l2(3) if do_l2 else None
                            act_o = gact.tile([P, H], HDT, tag="gact", name="act_o")
                            h_t = hpool.tile([P, H], HDT, tag="h_t")
                            for hh in (1, 0):
                                sl = slice(hh * HF, (hh + 1) * HF)
                                if do_l1:
                                    nc.scalar.activation(act_o[0:BPC, sl],
                                                         pgA_o[:, sl], AF.Sigmoid,
                                                         bias=zb[0:BPC],
                                                         scale=INV_WSCALE)
                                if do_l2:
                                    nc.scalar.activation(act_o[BPC:P, sl],
                                                         pgB_o[:, sl], AF.Sigmoid,
                                                         bias=zb[BPC:P],
                                                         scale=INV_WSCALE)
                                nc.vector.tensor_mul(h_t[r0:r1, sl],
                                                     act_o[r0:r1, sl],
                                                     tanh_c[r0:r1, sl])
                            # next-step L1 gate-i preload rides before the
                            # transposes: independent PE work in the tail
                            if t + 1 < T:
                                pgn = psum_g.tile([BPC, H], F32, tag="pg",
                                                  name="pgA0")
                                for n2 in range(2):
                                    nc.tensor.matmul(
                                        pgn[:, n2 * 512:(n2 + 1) * 512],
                                        xTa[:, t + 1, :],
                                        W_combT[:, n2 * 512:(n2 + 1) * 512],
                                        start=True, stop=False,
                                        skip_group_check=True)
                                pg_next[0] = pgn

                            # h^T transposes + ring writes (reversed: chunk
                            # 0, needed first next step, lands last)
                            if t <= T:
                                for k in range(KC - 1, -1, -1):
                                    pt = psum_tr.tile([P, P], HDT, tag="htr")
                                    nc.tensor.transpose(
                                        pt[:, r0:r1],
                                        h_t[r0:r1, k * P:(k + 1) * P],
                                        identh[r0:r1, r0:r1])
                                    nc.vector.tensor_scalar_mul(
                                        hT8[:, k // 2, k % 2, s_w, r0:r1],
                                        pt[:, r0:r1], HSCALE)
                                    if do_l1:
                                        # bf16 h1^T ring for the A2 GEMM
                                        nc.scalar.activation(
                                            hTb[:, k, s_w, :], pt[:, 0:BPC],
                                            AF.Copy)
                            else:
                                # final step: h2_{T-1}^T in bf16 for decode
                                for k in range(KC - 1, -1, -1):
                                    pt = psum_tr.tile([P, P], HDT, tag="htr")
                                    nc.tensor.transpose(
                                        pt[:, 0:BPC],
                                        h_t[BPC:P, k * P:(k + 1) * P],
                                        identh[BPC:P, BPC:P])
                                    nc.scalar.activation(
                                        hT_last[:, k, :], pt[:, 0:BPC], AF.Copy)

                        # decode: out = h2_{T-1} @ dec_W^T + dec_b
                        pd = psum_g.tile([1, BPC], F32, tag="pg", name="pdec")
                        for k in range(KC):
                            nc.tensor.matmul(pd[:], decWT[:, k:k + 1],
                                             hT_last[:, k, :],
                                             start=(k == 0), stop=False)
                        nc.tensor.matmul(pd[:], decb_sb[:], ones_bpc[:],
                                         start=False, stop=True)
                        osb = misc.tile([1, BPC], F32, tag="osb")
                        nc.vector.tensor_copy(osb[:], pd[:])
                        nc.sync.dma_start(out.rearrange("b o -> o b"), osb[:])

    nc.compile()
    return nc


_cached_nc = None
_cached_fn = None  # (jitted shard_map fn, in_names, out_names, out_shapes, zeros)


def _build_jitted(nc):
    """Same lowering as bass2jax.run_bass_via_pjrt, but the jitted
    executable is cached so repeat kernel() calls skip recompilation."""
    import jax
    from jax.sharding import Mesh, PartitionSpec
    from jax.experimental.shard_map import shard_map
    from concourse import bass2jax, mybir as _mybir

    bass2jax.install_neuronx_cc_hook()
    partition_name = nc.partition_id_tensor.name if nc.partition_id_tensor else None
    in_names, out_names, out_avals, zero_outs = [], [], [], []
    for alloc in nc.m.functions[0].allocations:
        if not isinstance(alloc, _mybir.MemoryLocationSet):
            continue
        name = alloc.memorylocations[0].name
        if alloc.kind == "ExternalInput":
            if name != partition_name:
                in_names.append(name)
        elif alloc.kind == "ExternalOutput":
            shape = tuple(alloc.tensor_shape)
            dtype = _mybir.dt.np(alloc.dtype)
            out_names.append(name)
            out_avals.append(jax.core.ShapedArray(shape, dtype))
            zero_outs.append(np.zeros(shape, dtype))
    n_params = len(in_names)
    n_outs = len(out_avals)
    all_in_names = list(in_names) + list(out_names)
    if partition_name is not None:
        all_in_names.append(partition_name)
    donate = tuple(range(n_params, n_params + n_outs))

    def _body(*args):
        operands = list(args)
        if partition_name is not None:
            operands.append(bass2jax.partition_id_tensor())
        outs = bass2jax._bass_exec_p.bind(
            *operands,
            out_avals=tuple(out_avals),
            in_names=tuple(all_in_names),
            out_names=tuple(out_names),
            lowering_input_output_aliases=(),
            sim_require_finite=True,
            sim_require_nnan=True,
            nc=nc,
        )
        return tuple(outs)

    devices = jax.devices()[:N_CORES]
    mesh = Mesh(np.asarray(devices), ("core",))
    in_specs = (PartitionSpec("core"),) * (n_params + n_outs)
    out_specs = (PartitionSpec("core"),) * n_outs
    fn = jax.jit(
        shard_map(_body, mesh=mesh, in_specs=in_specs, out_specs=out_specs,
                  check_rep=False),
        donate_argnums=donate, keep_unused=True,
    )
    out_shapes = [a.shape for a in out_avals]
    return fn, in_names, out_names, out_shapes, zero_outs


_dev_cache = {}  # name -> (digest, device_array)


def _to_device(name, arr):
    """Replicate-concat a weight to all cores and keep it on device across
    calls (keyed by content hash) so repeat kernel() calls only ship x."""
    import hashlib
    import jax
    d = hashlib.blake2b(arr.tobytes(), digest_size=16).digest()
    hit = _dev_cache.get(name)
    if hit is not None and hit[0] == d:
        return hit[1]
    conc = np.concatenate([arr] * N_CORES, axis=0)
    darr = jax.device_put(conc)
    _dev_cache[name] = (d, darr)
    return darr


def kernel(**inputs):
    global _cached_nc, _cached_fn
    if _cached_nc is None:
        _cached_nc = build(100)
        _cached_fn = _build_jitted(_cached_nc)
    fn, in_names, out_names, out_shapes, zero_outs = _cached_fn
    ins = {k: np.ascontiguousarray(np.asarray(v, dtype=np.float32))
           for k, v in inputs.items()}
    concat_in = []
    for name in in_names:
        if name == "x":
            concat_in.append(ins["x"])  # already [512, T, IN]; axis0 shards
        else:
            concat_in.append(_to_device(name, ins[name]))
    i = out_names.index("out")
    last_err = None
    for attempt in range(3):
        try:
            concat_zeros = [np.zeros((N_CORES * z.shape[0], *z.shape[1:]), z.dtype)
                            for z in zero_outs]
            out_arrs = fn(*concat_in, *concat_zeros)
            outp = np.asarray(out_arrs[i]).reshape(B, 1)
            return outp.astype(np.float32)
        except Exception as e:  # transient NRT_EXEC_UNIT_UNRECOVERABLE etc.
            last_err = e
            _dev_cache.clear()
            concat_in = []
            for name in in_names:
                if name == "x":
                    concat_in.append(ins["x"])
                else:
                    concat_in.append(_to_device(name, ins[name]))
    raise last_err


# revision 25
# speedup vs baseline: 1.0093x; 1.0093x over previous
"""Trainium2 Bass kernel for nn_Discriminator (2-layer LSTM, B=512 T=100 H=1024).

Strategy: data-parallel over batch across 8 cores (B=64 per core), with
both LSTM layers FUSED into one software-pipelined loop: macro-step t
computes layer-1 cell for time t and layer-2 cell for time t-2 (lag 2).
Gate preactivations accumulate in per-layer [64, 1024] PSUM tiles
(DoubleRow matmuls can only write PSUM partition 0), but the
activations write into ONE partition-stacked SBUF tile (L1 -> rows
0:64, L2 -> rows 64:128), so all downstream elementwise work (i*g, c
update, tanh, h) and the h^T transposes run once on [128, .] tiles for
both layers, and the PE always has >20us of mostly-independent matmul
work per step to hide the serial activation chain.

  - Recurrence products (h1@W_hh1^T, h2@W_hh2^T) are fp8e4m3 DoubleRow
    matmuls reading a shared fp8 h^T ring (cols 0:64 = h1, 64:128 =
    h2).  Weights are scaled x32 and h x16 before the fp8 cast
    (U(+-0.031) weights / small h are subnormal in e4m3 otherwise ->
    10-20% quantization error) and descaled for free via the
    activation `scale` operand.  Measured on HW: DR gives no
    per-instruction speedup (cost = N_out cycles regardless), but
    doubles K per instruction, which is what makes M=64 recurrences
    run at the same MAC rate as M=128 bf16 GEMMs.
  - The layer-2 input product h1@W_ih2^T MUST be bf16 on both operands
    (any fp8 operand pushes total rel err past the 2e-2 gate).  It is
    batched 2 timesteps at a time as an M=128 bf16 GEMM from a bf16
    h1^T ring (lag-2 makes both steps' h1 available), evicted
    PSUM->SBUF with the layer-2 bias added, then DVE-preset into the
    gate PSUM before the W_hh2 DoubleRow matmuls accumulate onto it
    (start=False onto DVE-written PSUM is legal).
  - Layer-1's input projection collapses through the encoder:
    W_comb = W_ih1 @ enc_W^T, and the per-step preload x_t @ W_comb^T
    rides a K=35 matmul whose lhsT carries [x_t ; ones]: the combined
    layer-1 bias lands with the projection.
  - Weight builds evict transposed k-pairs ([128, 2, 128]) in single
    DVE/Act ops -- evictions, not DMA, gate the prologue.
  - No DRAM scratch at all; HBM traffic is weights + x (~67MB/core,
    vs 199MB for the phase-separated baseline).  4.13ms -> 3.54ms.
"""

import numpy as np

import concourse.bass as bass
import concourse.tile as tile
import concourse.mybir as mybir
from concourse import bacc
from concourse.bass_utils import run_bass_kernel_spmd
from concourse.masks import make_identity

F32 = mybir.dt.float32
F32R = mybir.dt.float32r
BF16 = mybir.dt.bfloat16
FP8 = mybir.dt.float8e4
AF = mybir.ActivationFunctionType
DR = mybir.MatmulPerfMode.DoubleRow

N_CORES = 8
B, IN, H = 512, 34, 1024
G = 4 * H                 # 4096
BPC = B // N_CORES        # 64 batch rows per core
P = 128
KC = H // P               # 8 contraction chunks
KPF = KC // 2             # 4 fp8 k-pairs
NSLOT = 4                 # h^T ring depth
HDT = BF16
HF = 512                  # half of H for the split serial tail

WSCALE = 32.0             # fp8 weight pre-scale (exact power of 2)
HSCALE = 16.0             # fp8 h-ring pre-scale (fixes subnormal h)
INV_WSCALE = 1.0 / (WSCALE * HSCALE)


def _build_weight_T8(nc, w_dram, w_T8, identr, wrow, wtr_ps):
    """Transpose w_dram [G, H] into resident fp8 SBUF tile
    w_T8 [128, KPF, 2, G] (k-pair packed for DoubleRow), scaled by
    WSCALE in the PSUM->SBUF eviction (the PE transpose datapath
    ignores the identity operand's values, so scaling must not ride
    the transpose itself)."""
    n_row_tiles = w_dram.shape[0] // P  # 32
    for r in range(n_row_tiles):
        wt = wrow.tile([P, H], F32R, tag="wrow")
        nc.sync.dma_start(wt[:], w_dram[r * P:(r + 1) * P, :].bitcast(F32R))
        for kp in range(KPF):
            # two chunk transposes into one tile, ONE k-pair eviction
            # (evictions gate the build; halve their count)
            pt = wtr_ps.tile([P, 2, P], F32R, tag="wtr")
            for e in range(2):
                c = 2 * kp + e
                nc.tensor.transpose(pt[:, e, :], wt[:, c * P:(c + 1) * P],
                                    identr[:])
            dst = w_T8[:, kp, :, r * P:(r + 1) * P]
            # gpsimd cannot touch PSUM; alternate vector / scalar(Copy)
            if kp % 2 == 0:
                nc.vector.tensor_scalar_mul(dst, pt[:], WSCALE)
            else:
                nc.scalar.activation(dst, pt[:], AF.Copy, scale=WSCALE)


def build(T=100):
    nc = bacc.Bacc("TRN2", target_bir_lowering=False, debug=False,
                   num_devices=N_CORES)

    x = nc.dram_tensor("x", [BPC, T, IN], F32, kind="ExternalInput").ap()
    enc_W = nc.dram_tensor("enc_W", [H, IN], F32, kind="ExternalInput").ap()
    enc_b = nc.dram_tensor("enc_b", [H], F32, kind="ExternalInput").ap()
    W_ih1 = nc.dram_tensor("W_ih1", [G, H], F32, kind="ExternalInput").ap()
    W_hh1 = nc.dram_tensor("W_hh1", [G, H], F32, kind="ExternalInput").ap()
    b_ih1 = nc.dram_tensor("b_ih1", [G], F32, kind="ExternalInput").ap()
    b_hh1 = nc.dram_tensor("b_hh1", [G], F32, kind="ExternalInput").ap()
    W_ih2 = nc.dram_tensor("W_ih2", [G, H], F32, kind="ExternalInput").ap()
    W_hh2 = nc.dram_tensor("W_hh2", [G, H], F32, kind="ExternalInput").ap()
    b_ih2 = nc.dram_tensor("b_ih2", [G], F32, kind="ExternalInput").ap()
    b_hh2 = nc.dram_tensor("b_hh2", [G], F32, kind="ExternalInput").ap()
    dec_W = nc.dram_tensor("dec_W", [1, H], F32, kind="ExternalInput").ap()
    dec_b = nc.dram_tensor("dec_b", [1], F32, kind="ExternalInput").ap()
    out = nc.dram_tensor("out", [BPC, 1], F32, kind="ExternalOutput").ap()

    with tile.TileContext(nc) as tc:
        with tc.tile_pool(name="persist", bufs=1) as persist, \
             tc.tile_pool(name="state", bufs=1) as state, \
             tc.tile_pool(name="misc", bufs=1) as misc:

            ident = persist.tile([P, P], F32, tag="ident")
            make_identity(nc, ident[:])
            identr = persist.tile([P, P], F32R, tag="identr")
            nc.vector.tensor_copy(identr[:], ident[:])
            identh = persist.tile([P, P], HDT, tag="identh")
            nc.vector.tensor_copy(identh[:], ident[:])
            zb = persist.tile([P, 1], F32, tag="zero_bias")
            nc.gpsimd.memset(zb[:], 0.0)

            # layer-1 input-side operands, pre-scaled by WSCALE so the
            # PSUM accumulation matches the fp8 DR product scale:
            #   W_combT [35, G]: rows 0:34 = 32*(W_ih1@enc_W)^T,
            #     row 34 = 32*(enc_b@W_ih1^T + b_ih1 + b_hh1)
            #   xTa [35, T, 64]: per step t: [x_t^T ; ones]
            W_combT = persist.tile([IN + 1, G], BF16, tag="W_combT")
            xTa = persist.tile([IN + 1, T, BPC], BF16, tag="xTa")
            ones1 = persist.tile([1, P], F32R, tag="ones1")
            nc.gpsimd.memset(ones1[:].bitcast(F32), 1.0)
            # layer-2 bias broadcast [128, G], pre-scaled by WSCALE*HSCALE
            bias128_2 = persist.tile([P, G], BF16, tag="bias128_2")

            # ============ Phase E: xTa ============
            with nc.named_scope("phaseE"):
                with tc.tile_pool(name="e_sb", bufs=1) as e_sb, \
                     tc.tile_pool(name="e_ps", bufs=3, space="PSUM") as e_ps:
                    onesrow = e_sb.tile([1, T, BPC], BF16, tag="onesrow")
                    nc.gpsimd.memset(onesrow[:], 1.0)
                    nc.sync.dma_start(xTa[IN:IN + 1, :, :], onesrow[:])
                    # one bulk contiguous DMA of all of x (13.6KB/partition)
                    xall = e_sb.tile([BPC, T, IN], F32R, tag="xall")
                    nc.sync.dma_start(xall[:], x.bitcast(F32R))
                    for m in range(T // 2):
                        pt = e_ps.tile([IN, P], F32R, tag="xtr")
                        for e in range(2):
                            nc.tensor.transpose(
                                pt[:, e * BPC:(e + 1) * BPC],
                                xall[:, 2 * m + e, :], identr[0:BPC, 0:BPC])
                        nc.vector.tensor_copy(xTa[0:IN, 2 * m, :], pt[:, 0:BPC])
                        nc.scalar.activation(xTa[0:IN, 2 * m + 1, :], pt[:, BPC:P], AF.Copy)

            # ============ W_combT build (incremental, scaled) ============
            with nc.named_scope("build_Wcomb"):
                with tc.tile_pool(name="wc_sb", bufs=1) as wc_sb, \
                     tc.tile_pool(name="wc_row", bufs=6) as wc_row, \
                     tc.tile_pool(name="wc_st", bufs=2) as wc_st, \
                     tc.tile_pool(name="wc_ps", bufs=2, space="PSUM") as wc_ps, \
                     tc.tile_pool(name="wc_ps2", bufs=1, space="PSUM") as wc_ps2:
                    encwb = wc_sb.tile([P, KC, IN], F32R, tag="encwb")
                    nc.sync.dma_start(
                        encwb[:],
                        enc_W.rearrange("(c p) f -> p c f", p=P).bitcast(F32R))
                    encb_k = wc_sb.tile([P, KC], F32R, tag="encb_k")
                    nc.sync.dma_start(
                        encb_k[:],
                        enc_b.rearrange("(c p) -> p c", p=P).bitcast(F32R))
                    # WSCALE rides the encoder-side operands of the
                    # W_comb contraction (transposes don't scale)
                    nc.vector.tensor_scalar_mul(encwb[:], encwb[:], WSCALE * HSCALE)
                    nc.vector.tensor_scalar_mul(encb_k[:], encb_k[:], WSCALE * HSCALE)
                    brow1 = wc_sb.tile([1, G], BF16, tag="brow1")
                    bsum1 = wc_sb.tile([1, G], F32, tag="bsum1")
                    # pre-scaled bias sums; transient pool so the [1, G]
                    # f32 scratch frees before the weight staging runs
                    with tc.tile_pool(name="wc_tmp", bufs=1) as wc_tmp:
                        tA = wc_tmp.tile([1, G], F32, tag="tA")
                        nc.sync.dma_start(tA[:], b_ih1[None, :])
                        tB = wc_tmp.tile([1, G], F32, tag="tB")
                        nc.sync.dma_start(tB[:], b_hh1[None, :])
                        nc.vector.tensor_add(bsum1[:], tA[:], tB[:])
                        nc.gpsimd.tensor_scalar_mul(bsum1[:], bsum1[:], WSCALE * HSCALE)
                        tA = wc_tmp.tile([1, G], F32, tag="tA")
                        nc.sync.dma_start(tA[:], b_ih2[None, :])
                        tB = wc_tmp.tile([1, G], F32, tag="tB")
                        nc.sync.dma_start(tB[:], b_hh2[None, :])
                        nc.vector.tensor_add(tA[:], tA[:], tB[:])
                        nc.gpsimd.tensor_scalar_mul(tA[:], tA[:],
                                                    WSCALE * HSCALE)
                        brow2 = wc_tmp.tile([1, G], F32R, tag="brow2")
                        nc.vector.tensor_copy(brow2[:], tA[:])
                        for n in range(8):
                            slb = slice(n * 512, (n + 1) * 512)
                            pbb = wc_ps.tile([P, 512], F32, tag="pbb")
                            nc.tensor.matmul(pbb[:], ones1[:], brow2[:, slb],
                                             start=True, stop=True)
                            nc.vector.tensor_copy(bias128_2[:, slb], pbb[:])
                    # groups of 4 row-chunks = 512 G columns
                    for grp in range(G // 512):
                        wstage = wc_st.tile([P, KC, 512], F32R, tag="wstage")
                        for rr in range(4):
                            r = grp * 4 + rr
                            wt = wc_row.tile([P, H], F32R, tag="wcrow")
                            nc.sync.dma_start(
                                wt[:], W_ih1[r * P:(r + 1) * P, :].bitcast(F32R))
                            for kp in range(KPF):
                                ptr = wc_ps.tile([P, 2, P], F32R, tag="wctr")
                                for e in range(2):
                                    c = 2 * kp + e
                                    nc.tensor.transpose(
                                        ptr[:, e, :], wt[:, c * P:(c + 1) * P],
                                        identr[:])
                                dstw = wstage[:, 2 * kp:2 * kp + 2,
                                              rr * P:(rr + 1) * P]
                                if kp % 2 == 0:
                                    nc.vector.tensor_copy(dstw, ptr[:])
                                else:
                                    nc.scalar.activation(dstw, ptr[:], AF.Copy)
                        pb = wc_ps2.tile([IN, 512], F32, tag="wcpb")
                        pbias = wc_ps2.tile([1, 512], F32, tag="wcpbias")
                        for k in range(KC):
                            nc.tensor.matmul(pb[:], encwb[:, k, :],
                                             wstage[:, k, :],
                                             start=(k == 0), stop=(k == KC - 1))
                        for k in range(KC):
                            nc.tensor.matmul(pbias[:], encb_k[:, k:k + 1],
                                             wstage[:, k, :],
                                             start=(k == 0), stop=(k == KC - 1))
                        sl = slice(grp * 512, (grp + 1) * 512)
                        nc.vector.tensor_copy(W_combT[0:IN, sl], pb[:])
                        nc.vector.tensor_add(brow1[:, sl], pbias[:], bsum1[:, sl])
                    # bias row rides as contraction row 34 (DMA can hit
                    # the unaligned partition offset)
                    nc.sync.dma_start(W_combT[IN:IN + 1, :], brow1[:])

            # ============ fp8 weight builds (all resident) ============
            with tc.tile_pool(name="wpool", bufs=1) as wpool:
                w1 = wpool.tile([P, KPF, 2, G], FP8, tag="Whh1")
                w2h = wpool.tile([P, KPF, 2, G], FP8, tag="Whh2")
                w2b = wpool.tile([P, KC, G], BF16, tag="Wih2b")
                with nc.named_scope("build_W8"):
                    with tc.tile_pool(name="wrow1", bufs=6) as wrow, \
                         tc.tile_pool(name="wtr_ps1", bufs=3, space="PSUM") as wtr_ps:
                        _build_weight_T8(nc, W_hh1, w1, identr, wrow, wtr_ps)
                        _build_weight_T8(nc, W_hh2, w2h, identr, wrow, wtr_ps)
                        # W_ih2^T in bf16 (x512) for the batched A2 GEMM
                        for r in range(G // P):
                            wt = wrow.tile([P, H], F32R, tag="wrow")
                            nc.sync.dma_start(
                                wt[:], W_ih2[r * P:(r + 1) * P, :].bitcast(F32R))
                            for kp in range(KPF):
                                pt = wtr_ps.tile([P, 2, P], F32R, tag="wtr")
                                for e in range(2):
                                    c = 2 * kp + e
                                    nc.tensor.transpose(
                                        pt[:, e, :], wt[:, c * P:(c + 1) * P],
                                        identr[:])
                                dst = w2b[:, 2 * kp:2 * kp + 2,
                                          r * P:(r + 1) * P]
                                if kp % 2 == 0:
                                    nc.vector.tensor_scalar_mul(dst, pt[:],
                                                                WSCALE * HSCALE)
                                else:
                                    nc.scalar.activation(dst, pt[:], AF.Copy,
                                                         scale=WSCALE * HSCALE)

                # persistent state
                hT8 = state.tile([P, KPF, 2, NSLOT, P], FP8, tag="hT8_ring")
                hTb = state.tile([P, KC, 2, 2, BPC], HDT, tag="hTb_ring")
                c_st = state.tile([P, H], F32, tag="c_stack")
                nc.gpsimd.memset(hT8[:].bitcast(mybir.dt.uint8), 0.0)
                nc.gpsimd.memset(hTb[:].bitcast(mybir.dt.uint16), 0.0)
                nc.gpsimd.memset(c_st[:], 0.0)

                # decode operands
                decWT_f = misc.tile([P, KC], F32, tag="decWT_f")
                nc.sync.dma_start(decWT_f[:], dec_W.rearrange("o (c p) -> p (c o)", p=P))
                decWT = misc.tile([P, KC], HDT, tag="decWT")
                nc.vector.tensor_copy(decWT[:], decWT_f[:])
                decb_f = misc.tile([1, 1], F32, tag="decb_f")
                nc.sync.dma_start(decb_f[:], dec_b[None, :])
                decb_sb = misc.tile([1, 1], HDT, tag="decb")
                nc.vector.tensor_copy(decb_sb[:], decb_f[:])
                ones_f = misc.tile([1, BPC], F32, tag="ones_f")
                nc.gpsimd.memset(ones_f[:], 1.0)
                ones_bpc = misc.tile([1, BPC], HDT, tag="ones_bpc")
                nc.vector.tensor_copy(ones_bpc[:], ones_f[:])
                hT_last = misc.tile([P, KC, BPC], HDT, tag="hT_last")

                # ============ fused recurrence loop ============
                with nc.named_scope("loop"):
                    with tc.tile_pool(name="l_g", bufs=4) as gact, \
                         tc.tile_pool(name="l_a2", bufs=2) as a2pool, \
                         tc.tile_pool(name="l_h", bufs=2) as hpool, \
                         tc.tile_pool(name="l_pg", bufs=3, space="PSUM") as psum_g, \
                         tc.tile_pool(name="l_ptr", bufs=2, space="PSUM") as psum_tr:
                        pg_next = {}
                        a2_cur = None
                        for t in range(T + 3):
                            do_l1 = t < T
                            do_l2 = t >= 3
                            r0 = 0 if do_l1 else BPC
                            r1 = P if do_l2 else BPC
                            s_r = (t - 1) % NSLOT
                            s_w = t % NSLOT
                            rh = BPC * ((t + 1) % 2)  # a2 row-half, L2-time t-3

                            def a2_block(chunks):
                                # batched bf16 A2 for L2-times (t-2, t-1),
                                # emitted in THIS step's tail where its h1^T
                                # inputs are >=1 step old (no PE wait) and the
                                # matmuls bridge the step-boundary bubble
                                pair = (t - 2) // 2
                                for chn in chunks:
                                    cs = slice(chn * 512, (chn + 1) * 512)
                                    pa = psum_tr.tile([P, 512], F32, tag="htr",
                                                      name="pa")
                                    for k in range(KC):
                                        nc.tensor.matmul(
                                            pa[:], hTb[:, k, pair % 2, :, :],
                                            w2b[:, k, cs],
                                            start=(k == 0), stop=(k == KC - 1),
                                            skip_group_check=True)
                                    nc.vector.tensor_add(a2_cur[:, cs], pa[:],
                                                         bias128_2[:, cs])

                            def mk_pgA(g_idx):
                                pgA = psum_g.tile([BPC, H], F32, tag="pg",
                                                  name=f"pgA{g_idx}")
                                for n2 in range(2):
                                    n = g_idx * 2 + n2
                                    nc.tensor.matmul(
                                        pgA[:, n2 * 512:(n2 + 1) * 512],
                                        xTa[:, t, :],
                                        W_combT[:, n * 512:(n + 1) * 512],
                                        start=True, stop=False,
                                        skip_group_check=True)
                                return pgA

                            def mm_l1(g_idx, pgA):
                                for n2 in range(2):
                                    n = g_idx * 2 + n2
                                    ch = slice(n2 * 512, (n2 + 1) * 512)
                                    wch = slice(n * 512, (n + 1) * 512)
                                    for kp in range(KPF):
                                        nc.tensor.matmul(
                                            pgA[:, ch],
                                            hT8[:, kp, :, s_r, 0:BPC],
                                            w1[:, kp, :, wch],
                                            start=False, stop=(kp == KPF - 1),
                                            perf_mode=DR,
                                            skip_group_check=True)

                            def mm_l2(g_idx):
                                pgB = psum_g.tile([BPC, H], F32, tag="pg",
                                                  name=f"pgB{g_idx}")
                                # a2 (+bias) preset via DVE, then accumulate
                                nc.vector.tensor_copy(
                                    pgB[:],
                                    a2_cur[rh:rh + BPC,
                                           g_idx * H:(g_idx + 1) * H])
                                for n2 in range(2):
                                    n = g_idx * 2 + n2
                                    ch = slice(n2 * 512, (n2 + 1) * 512)
                                    wch = slice(n * 512, (n + 1) * 512)
                                    for kp in range(KPF):
                                        nc.tensor.matmul(
                                            pgB[:, ch],
                                            hT8[:, kp, :, s_r, BPC:P],
                                            w2h[:, kp, :, wch],
                                            start=False, stop=(kp == KPF - 1),
                                            perf_mode=DR,
                                            skip_group_check=True)
                                return pgB

                            acts = {}

                            def do_gate(g_idx, func, name):
                                pgA = pg_next.pop(g_idx, None)
                                if do_l1:
                                    if pgA is None:
                                        pgA = mk_pgA(g_idx)
                                    mm_l1(g_idx, pgA)
                                pgB = mm_l2(g_idx) if do_l2 else None
                                at = gact.tile([P, H], HDT, tag="gact", name=name)
                                if do_l1:
                                    nc.scalar.activation(at[0:BPC], pgA[:], func,
                                                         bias=zb[0:BPC],
                                                         scale=INV_WSCALE)
                                if do_l2:
                                    nc.scalar.activation(at[BPC:P], pgB[:], func,
                                                         bias=zb[BPC:P],
                                                         scale=INV_WSCALE)
                                acts[g_idx] = at
                                return pgA, pgB

                            do_gate(0, AF.Sigmoid, "act_i")
                            do_gate(2, AF.Tanh, "act_g")
                            tmp = gact.tile([P, H], HDT, tag="gact", name="tmp")
                            nc.vector.tensor_mul(tmp[r0:r1], acts[0][r0:r1],
                                                 acts[2][r0:r1])

                            # gate f, then c update + tanh(c), in halves
                            pgA_f = pg_next.pop(1, None)
                            if do_l1:
                                if pgA_f is None:
                                    pgA_f = mk_pgA(1)
                                mm_l1(1, pgA_f)
                            pgB_f = mm_l2(1) if do_l2 else None
                            act_f = gact.tile([P, H], HDT, tag="gact", name="act_f")
                            tanh_c = gact.tile([P, H], HDT, tag="gact", name="tanh_c")
                            for hh in (1, 0):
                                sl = slice(hh * HF, (hh + 1) * HF)
                                if do_l1:
                                    nc.scalar.activation(act_f[0:BPC, sl],
                                                         pgA_f[:, sl], AF.Sigmoid,
                                                         bias=zb[0:BPC],
                                                         scale=INV_WSCALE)
                                if do_l2:
                                    nc.scalar.activation(act_f[BPC:P, sl],
                                                         pgB_f[:, sl], AF.Sigmoid,
                                                         bias=zb[BPC:P],
                                                         scale=INV_WSCALE)
                                nc.vector.tensor_mul(c_st[r0:r1, sl],
                                                     c_st[r0:r1, sl],
                                                     act_f[r0:r1, sl])
                                nc.vector.tensor_add(c_st[r0:r1, sl],
                                                     c_st[r0:r1, sl],
                                                     tmp[r0:r1, sl])
                                nc.scalar.activation(tanh_c[r0:r1, sl],
                                                     c_st[r0:r1, sl], AF.Tanh,
                                                     bias=zb[r0:r1])

                            # gate o + h, in halves
                            pgA_o = pg_next.pop(3, None)
                            if do_l1:
                                if pgA_o is None:
                                    pgA_o = mk_pgA(3)
                                mm_l1(3, pgA_o)
                            pgB_o = mm_l2(3) if do_l2 else None
                            act_o = gact.tile([P, H], HDT, tag="gact", name="act_o")
                            h_t = hpool.tile([P, H], HDT, tag="h_t")
                            for hh in (1, 0):
                                sl = slice(hh * HF, (hh + 1) * HF)
                                if do_l1:
                                    nc.scalar.activation(act_o[0:BPC, sl],
                                                         pgA_o[:, sl], AF.Sigmoid,
                                                         bias=zb[0:BPC],
                                                         scale=INV_WSCALE)
                                if do_l2:
                                    nc.scalar.activation(act_o[BPC:P, sl],
                                                         pgB_o[:, sl], AF.Sigmoid,
                                                         bias=zb[BPC:P],
                                                         scale=INV_WSCALE)
                                nc.vector.tensor_mul(h_t[r0:r1, sl],
                                                     act_o[r0:r1, sl],
                                                     tanh_c[r0:r1, sl])
                            # next-step L1 gate-i preload rides before the
                            # transposes: independent PE work in the tail
                            if t + 1 < T:
                                pgn = psum_g.tile([BPC, H], F32, tag="pg",
                                                  name="pgA0")
                                for n2 in range(2):
                                    nc.tensor.matmul(
                                        pgn[:, n2 * 512:(n2 + 1) * 512],
                                        xTa[:, t + 1, :],
                                        W_combT[:, n2 * 512:(n2 + 1) * 512],
                                        start=True, stop=False,
                                        skip_group_check=True)
                                pg_next[0] = pgn

                            if 2 <= t <= T and t % 2 == 0:
                                a2_cur = a2pool.tile([P, G], HDT, tag="a2sb")
                                a2_block(range(0, 4))

                            # h^T transposes + ring writes (reversed: chunk
                            # 0, needed first next step, lands last)
                            if t <= T + 1:
                                for k in range(KC - 1, -1, -1):
                                    pt = psum_tr.tile([P, P], HDT, tag="htr")
                                    nc.tensor.transpose(
                                        pt[:, r0:r1],
                                        h_t[r0:r1, k * P:(k + 1) * P],
                                        identh[r0:r1, r0:r1])
                                    nc.vector.tensor_scalar_mul(
                                        hT8[:, k // 2, k % 2, s_w, r0:r1],
                                        pt[:, r0:r1], HSCALE)
                                    if do_l1:
                                        # bf16 h1^T pair-ring for the A2 GEMM
                                        nc.scalar.activation(
                                            hTb[:, k, (t // 2) % 2, t % 2, :],
                                            pt[:, 0:BPC], AF.Copy)
                                if 2 <= t <= T and t % 2 == 0:
                                    a2_block(range(4, 8))
                            else:
                                # final step: h2_{T-1}^T in bf16 for decode
                                for k in range(KC - 1, -1, -1):
                                    pt = psum_tr.tile([P, P], HDT, tag="htr")
                                    nc.tensor.transpose(
                                        pt[:, 0:BPC],
                                        h_t[BPC:P, k * P:(k + 1) * P],
                                        identh[BPC:P, BPC:P])
                                    nc.scalar.activation(
                                        hT_last[:, k, :], pt[:, 0:BPC], AF.Copy)

                        # decode: out = h2_{T-1} @ dec_W^T + dec_b
                        pd = psum_g.tile([1, BPC], F32, tag="pg", name="pdec")
                        for k in range(KC):
                            nc.tensor.matmul(pd[:], decWT[:, k:k + 1],
                                             hT_last[:, k, :],
                                             start=(k == 0), stop=False)
                        nc.tensor.matmul(pd[:], decb_sb[:], ones_bpc[:],
                                         start=False, stop=True)
                        osb = misc.tile([1, BPC], F32, tag="osb")
                        nc.vector.tensor_copy(osb[:], pd[:])
                        nc.sync.dma_start(out.rearrange("b o -> o b"), osb[:])

    nc.compile()
    return nc


_cached_nc = None
_cached_fn = None  # (jitted shard_map fn, in_names, out_names, out_shapes, zeros)


def _build_jitted(nc):
    """Same lowering as bass2jax.run_bass_via_pjrt, but the jitted
    executable is cached so repeat kernel() calls skip recompilation."""
    import jax
    from jax.sharding import Mesh, PartitionSpec
    from jax.experimental.shard_map import shard_map
    from concourse import bass2jax, mybir as _mybir

    bass2jax.install_neuronx_cc_hook()
    partition_name = nc.partition_id_tensor.name if nc.partition_id_tensor else None
    in_names, out_names, out_avals, zero_outs = [], [], [], []
    for alloc in nc.m.functions[0].allocations:
        if not isinstance(alloc, _mybir.MemoryLocationSet):
            continue
        name = alloc.memorylocations[0].name
        if alloc.kind == "ExternalInput":
            if name != partition_name:
                in_names.append(name)
        elif alloc.kind == "ExternalOutput":
            shape = tuple(alloc.tensor_shape)
            dtype = _mybir.dt.np(alloc.dtype)
            out_names.append(name)
            out_avals.append(jax.core.ShapedArray(shape, dtype))
            zero_outs.append(np.zeros(shape, dtype))
    n_params = len(in_names)
    n_outs = len(out_avals)
    all_in_names = list(in_names) + list(out_names)
    if partition_name is not None:
        all_in_names.append(partition_name)
    donate = tuple(range(n_params, n_params + n_outs))

    def _body(*args):
        operands = list(args)
        if partition_name is not None:
            operands.append(bass2jax.partition_id_tensor())
        outs = bass2jax._bass_exec_p.bind(
            *operands,
            out_avals=tuple(out_avals),
            in_names=tuple(all_in_names),
            out_names=tuple(out_names),
            lowering_input_output_aliases=(),
            sim_require_finite=True,
            sim_require_nnan=True,
            nc=nc,
        )
        return tuple(outs)

    devices = jax.devices()[:N_CORES]
    mesh = Mesh(np.asarray(devices), ("core",))
    in_specs = (PartitionSpec("core"),) * (n_params + n_outs)
    out_specs = (PartitionSpec("core"),) * n_outs
    fn = jax.jit(
        shard_map(_body, mesh=mesh, in_specs=in_specs, out_specs=out_specs,
                  check_rep=False),
        donate_argnums=donate, keep_unused=True,
    )
    out_shapes = [a.shape for a in out_avals]
    return fn, in_names, out_names, out_shapes, zero_outs


_dev_cache = {}  # name -> (digest, device_array)


def _to_device(name, arr):
    """Replicate-concat a weight to all cores and keep it on device across
    calls (keyed by content hash) so repeat kernel() calls only ship x."""
    import hashlib
    import jax
    d = hashlib.blake2b(arr.tobytes(), digest_size=16).digest()
    hit = _dev_cache.get(name)
    if hit is not None and hit[0] == d:
        return hit[1]
    conc = np.concatenate([arr] * N_CORES, axis=0)
    darr = jax.device_put(conc)
    _dev_cache[name] = (d, darr)
    return darr


def kernel(**inputs):
    global _cached_nc, _cached_fn
    if _cached_nc is None:
        _cached_nc = build(100)
        _cached_fn = _build_jitted(_cached_nc)
    fn, in_names, out_names, out_shapes, zero_outs = _cached_fn
    ins = {k: np.ascontiguousarray(np.asarray(v, dtype=np.float32))
           for k, v in inputs.items()}
    concat_in = []
    for name in in_names:
        if name == "x":
            concat_in.append(ins["x"])  # already [512, T, IN]; axis0 shards
        else:
            concat_in.append(_to_device(name, ins[name]))
    i = out_names.index("out")
    last_err = None
    for attempt in range(3):
        try:
            concat_zeros = [np.zeros((N_CORES * z.shape[0], *z.shape[1:]), z.dtype)
                            for z in zero_outs]
            out_arrs = fn(*concat_in, *concat_zeros)
            outp = np.asarray(out_arrs[i]).reshape(B, 1)
            return outp.astype(np.float32)
        except Exception as e:  # transient NRT_EXEC_UNIT_UNRECOVERABLE etc.
            last_err = e
            _dev_cache.clear()
            concat_in = []
            for name in in_names:
                if name == "x":
                    concat_in.append(ins["x"])
                else:
                    concat_in.append(_to_device(name, ins[name]))
    raise last_err


# revision 26
# speedup vs baseline: 1.0822x; 1.0723x over previous
"""Trainium2 Bass kernel for nn_Discriminator (2-layer LSTM, B=512 T=100 H=1024).

Strategy: data-parallel over batch across 8 cores (B=64 per core), with
both LSTM layers FUSED into one software-pipelined loop: macro-step t
computes layer-1 cell for time t and layer-2 cell for time t-2 (lag 2).
Gate preactivations accumulate in per-layer [64, 1024] PSUM tiles
(DoubleRow matmuls can only write PSUM partition 0), but the
activations write into ONE partition-stacked SBUF tile (L1 -> rows
0:64, L2 -> rows 64:128), so all downstream elementwise work (i*g, c
update, tanh, h) and the h^T transposes run once on [128, .] tiles for
both layers, and the PE always has >20us of mostly-independent matmul
work per step to hide the serial activation chain.

  - Recurrence products (h1@W_hh1^T, h2@W_hh2^T) are fp8e4m3 DoubleRow
    matmuls reading a shared fp8 h^T ring (cols 0:64 = h1, 64:128 =
    h2).  Weights are scaled x32 and h x16 before the fp8 cast
    (U(+-0.031) weights / small h are subnormal in e4m3 otherwise ->
    10-20% quantization error) and descaled for free via the
    activation `scale` operand.  Measured on HW: DR gives no
    per-instruction speedup (cost = N_out cycles regardless), but
    doubles K per instruction, which is what makes M=64 recurrences
    run at the same MAC rate as M=128 bf16 GEMMs.
  - The layer-2 input product h1@W_ih2^T MUST be bf16 on both operands
    (any fp8 operand pushes total rel err past the 2e-2 gate).  It is
    batched 2 timesteps at a time as an M=128 bf16 GEMM from a bf16
    h1^T ring (lag-2 makes both steps' h1 available), evicted
    PSUM->SBUF with the layer-2 bias added, then DVE-preset into the
    gate PSUM before the W_hh2 DoubleRow matmuls accumulate onto it
    (start=False onto DVE-written PSUM is legal).
  - Layer-1's input projection collapses through the encoder:
    W_comb = W_ih1 @ enc_W^T, and the per-step preload x_t @ W_comb^T
    rides a K=35 matmul whose lhsT carries [x_t ; ones]: the combined
    layer-1 bias lands with the projection.
  - Weight builds evict transposed k-pairs ([128, 2, 128]) in single
    DVE/Act ops -- evictions, not DMA, gate the prologue.
  - No DRAM scratch at all; HBM traffic is weights + x (~67MB/core,
    vs 199MB for the phase-separated baseline).  4.13ms -> 3.54ms.
"""

import numpy as np

import concourse.bass as bass
import concourse.tile as tile
import concourse.mybir as mybir
from concourse import bacc
from concourse.bass_utils import run_bass_kernel_spmd
from concourse.masks import make_identity

F32 = mybir.dt.float32
F32R = mybir.dt.float32r
BF16 = mybir.dt.bfloat16
FP8 = mybir.dt.float8e4
AF = mybir.ActivationFunctionType
DR = mybir.MatmulPerfMode.DoubleRow

N_CORES = 8
B, IN, H = 512, 34, 1024
G = 4 * H                 # 4096
BPC = B // N_CORES        # 64 batch rows per core
P = 128
KC = H // P               # 8 contraction chunks
KPF = KC // 2             # 4 fp8 k-pairs
NSLOT = 4                 # h^T ring depth
HDT = BF16
HF = 512                  # half of H for the split serial tail

WSCALE = 32.0             # fp8 weight pre-scale (exact power of 2)
HSCALE = 16.0             # fp8 h-ring pre-scale (fixes subnormal h)
INV_WSCALE = 1.0 / (WSCALE * HSCALE)


def _build_weight_T8(nc, w_dram, w_T8, identr, wrow, wtr_ps):
    """Transpose w_dram [G, H] into resident fp8 SBUF tile
    w_T8 [128, KPF, 2, G] (k-pair packed for DoubleRow), scaled by
    WSCALE in the PSUM->SBUF eviction (the PE transpose datapath
    ignores the identity operand's values, so scaling must not ride
    the transpose itself)."""
    n_row_tiles = w_dram.shape[0] // P  # 32
    for r in range(n_row_tiles):
        wt = wrow.tile([P, H], F32R, tag="wrow")
        nc.sync.dma_start(wt[:], w_dram[r * P:(r + 1) * P, :].bitcast(F32R))
        for kp in range(KPF):
            # two chunk transposes into one tile, ONE k-pair eviction
            # (evictions gate the build; halve their count)
            pt = wtr_ps.tile([P, 2, P], F32R, tag="wtr")
            for e in range(2):
                c = 2 * kp + e
                nc.tensor.transpose(pt[:, e, :], wt[:, c * P:(c + 1) * P],
                                    identr[:])
            dst = w_T8[:, kp, :, r * P:(r + 1) * P]
            # gpsimd cannot touch PSUM; alternate vector / scalar(Copy)
            if kp % 2 == 0:
                nc.vector.tensor_scalar_mul(dst, pt[:], WSCALE)
            else:
                nc.scalar.activation(dst, pt[:], AF.Copy, scale=WSCALE)


def build(T=100):
    nc = bacc.Bacc("TRN2", target_bir_lowering=False, debug=False,
                   num_devices=N_CORES)

    x = nc.dram_tensor("x", [BPC, T, IN], F32, kind="ExternalInput").ap()
    enc_W = nc.dram_tensor("enc_W", [H, IN], F32, kind="ExternalInput").ap()
    enc_b = nc.dram_tensor("enc_b", [H], F32, kind="ExternalInput").ap()
    W_ih1 = nc.dram_tensor("W_ih1", [G, H], F32, kind="ExternalInput").ap()
    W_hh1 = nc.dram_tensor("W_hh1", [G, H], F32, kind="ExternalInput").ap()
    b_ih1 = nc.dram_tensor("b_ih1", [G], F32, kind="ExternalInput").ap()
    b_hh1 = nc.dram_tensor("b_hh1", [G], F32, kind="ExternalInput").ap()
    W_ih2 = nc.dram_tensor("W_ih2", [G, H], F32, kind="ExternalInput").ap()
    W_hh2 = nc.dram_tensor("W_hh2", [G, H], F32, kind="ExternalInput").ap()
    b_ih2 = nc.dram_tensor("b_ih2", [G], F32, kind="ExternalInput").ap()
    b_hh2 = nc.dram_tensor("b_hh2", [G], F32, kind="ExternalInput").ap()
    dec_W = nc.dram_tensor("dec_W", [1, H], F32, kind="ExternalInput").ap()
    dec_b = nc.dram_tensor("dec_b", [1], F32, kind="ExternalInput").ap()
    out = nc.dram_tensor("out", [BPC, 1], F32, kind="ExternalOutput").ap()

    with tile.TileContext(nc) as tc:
        with tc.tile_pool(name="persist", bufs=1) as persist, \
             tc.tile_pool(name="state", bufs=1) as state, \
             tc.tile_pool(name="misc", bufs=1) as misc:

            ident = persist.tile([P, P], F32, tag="ident")
            make_identity(nc, ident[:])
            identr = persist.tile([P, P], F32R, tag="identr")
            nc.vector.tensor_copy(identr[:], ident[:])
            identh = persist.tile([P, P], HDT, tag="identh")
            nc.vector.tensor_copy(identh[:], ident[:])
            zb = persist.tile([P, 1], F32, tag="zero_bias")
            nc.gpsimd.memset(zb[:], 0.0)

            # layer-1 input-side operands, pre-scaled by WSCALE so the
            # PSUM accumulation matches the fp8 DR product scale:
            #   W_combT [35, G]: rows 0:34 = 32*(W_ih1@enc_W)^T,
            #     row 34 = 32*(enc_b@W_ih1^T + b_ih1 + b_hh1)
            #   xTa [35, T, 64]: per step t: [x_t^T ; ones]
            W_combT = persist.tile([IN + 1, G], BF16, tag="W_combT")
            xTa = persist.tile([IN + 1, T, BPC], BF16, tag="xTa")
            ones1 = persist.tile([1, P], F32R, tag="ones1")
            nc.gpsimd.memset(ones1[:].bitcast(F32), 1.0)
            # layer-2 bias broadcast [128, G], pre-scaled by WSCALE*HSCALE
            bias128_2 = persist.tile([P, G], BF16, tag="bias128_2")

            # ============ Phase E: xTa ============
            with nc.named_scope("phaseE"):
                with tc.tile_pool(name="e_sb", bufs=1) as e_sb, \
                     tc.tile_pool(name="e_ps", bufs=3, space="PSUM") as e_ps:
                    onesrow = e_sb.tile([1, T, BPC], BF16, tag="onesrow")
                    nc.gpsimd.memset(onesrow[:], 1.0)
                    nc.sync.dma_start(xTa[IN:IN + 1, :, :], onesrow[:])
                    # one bulk contiguous DMA of all of x (13.6KB/partition)
                    xall = e_sb.tile([BPC, T, IN], F32R, tag="xall")
                    nc.sync.dma_start(xall[:], x.bitcast(F32R))
                    for m in range(T // 2):
                        pt = e_ps.tile([IN, P], F32R, tag="xtr")
                        for e in range(2):
                            nc.tensor.transpose(
                                pt[:, e * BPC:(e + 1) * BPC],
                                xall[:, 2 * m + e, :], identr[0:BPC, 0:BPC])
                        nc.vector.tensor_copy(xTa[0:IN, 2 * m, :], pt[:, 0:BPC])
                        nc.scalar.activation(xTa[0:IN, 2 * m + 1, :], pt[:, BPC:P], AF.Copy)

            # ============ W_combT build (incremental, scaled) ============
            with nc.named_scope("build_Wcomb"):
                with tc.tile_pool(name="wc_sb", bufs=1) as wc_sb, \
                     tc.tile_pool(name="wc_row", bufs=6) as wc_row, \
                     tc.tile_pool(name="wc_st", bufs=2) as wc_st, \
                     tc.tile_pool(name="wc_ps", bufs=2, space="PSUM") as wc_ps, \
                     tc.tile_pool(name="wc_ps2", bufs=1, space="PSUM") as wc_ps2:
                    encwb = wc_sb.tile([P, KC, IN], F32R, tag="encwb")
                    nc.sync.dma_start(
                        encwb[:],
                        enc_W.rearrange("(c p) f -> p c f", p=P).bitcast(F32R))
                    encb_k = wc_sb.tile([P, KC], F32R, tag="encb_k")
                    nc.sync.dma_start(
                        encb_k[:],
                        enc_b.rearrange("(c p) -> p c", p=P).bitcast(F32R))
                    # WSCALE rides the encoder-side operands of the
                    # W_comb contraction (transposes don't scale)
                    nc.vector.tensor_scalar_mul(encwb[:], encwb[:], WSCALE * HSCALE)
                    nc.vector.tensor_scalar_mul(encb_k[:], encb_k[:], WSCALE * HSCALE)
                    brow1 = wc_sb.tile([1, G], BF16, tag="brow1")
                    bsum1 = wc_sb.tile([1, G], F32, tag="bsum1")
                    # pre-scaled bias sums; transient pool so the [1, G]
                    # f32 scratch frees before the weight staging runs
                    with tc.tile_pool(name="wc_tmp", bufs=1) as wc_tmp:
                        tA = wc_tmp.tile([1, G], F32, tag="tA")
                        nc.sync.dma_start(tA[:], b_ih1[None, :])
                        tB = wc_tmp.tile([1, G], F32, tag="tB")
                        nc.sync.dma_start(tB[:], b_hh1[None, :])
                        nc.vector.tensor_add(bsum1[:], tA[:], tB[:])
                        nc.gpsimd.tensor_scalar_mul(bsum1[:], bsum1[:], WSCALE * HSCALE)
                        tA = wc_tmp.tile([1, G], F32, tag="tA")
                        nc.sync.dma_start(tA[:], b_ih2[None, :])
                        tB = wc_tmp.tile([1, G], F32, tag="tB")
                        nc.sync.dma_start(tB[:], b_hh2[None, :])
                        nc.vector.tensor_add(tA[:], tA[:], tB[:])
                        nc.gpsimd.tensor_scalar_mul(tA[:], tA[:],
                                                    WSCALE * HSCALE)
                        brow2 = wc_tmp.tile([1, G], F32R, tag="brow2")
                        nc.vector.tensor_copy(brow2[:], tA[:])
                        for n in range(8):
                            slb = slice(n * 512, (n + 1) * 512)
                            pbb = wc_ps.tile([P, 512], F32, tag="pbb")
                            nc.tensor.matmul(pbb[:], ones1[:], brow2[:, slb],
                                             start=True, stop=True)
                            nc.vector.tensor_copy(bias128_2[:, slb], pbb[:])
                    # groups of 4 row-chunks = 512 G columns
                    for grp in range(G // 512):
                        wstage = wc_st.tile([P, KC, 512], F32R, tag="wstage")
                        for rr in range(4):
                            r = grp * 4 + rr
                            wt = wc_row.tile([P, H], F32R, tag="wcrow")
                            nc.sync.dma_start(
                                wt[:], W_ih1[r * P:(r + 1) * P, :].bitcast(F32R))
                            for kp in range(KPF):
                                ptr = wc_ps.tile([P, 2, P], F32R, tag="wctr")
                                for e in range(2):
                                    c = 2 * kp + e
                                    nc.tensor.transpose(
                                        ptr[:, e, :], wt[:, c * P:(c + 1) * P],
                                        identr[:])
                                dstw = wstage[:, 2 * kp:2 * kp + 2,
                                              rr * P:(rr + 1) * P]
                                if kp % 2 == 0:
                                    nc.vector.tensor_copy(dstw, ptr[:])
                                else:
                                    nc.scalar.activation(dstw, ptr[:], AF.Copy)
                        pb = wc_ps2.tile([IN, 512], F32, tag="wcpb")
                        pbias = wc_ps2.tile([1, 512], F32, tag="wcpbias")
                        for k in range(KC):
                            nc.tensor.matmul(pb[:], encwb[:, k, :],
                                             wstage[:, k, :],
                                             start=(k == 0), stop=(k == KC - 1))
                        for k in range(KC):
                            nc.tensor.matmul(pbias[:], encb_k[:, k:k + 1],
                                             wstage[:, k, :],
                                             start=(k == 0), stop=(k == KC - 1))
                        sl = slice(grp * 512, (grp + 1) * 512)
                        nc.vector.tensor_copy(W_combT[0:IN, sl], pb[:])
                        nc.vector.tensor_add(brow1[:, sl], pbias[:], bsum1[:, sl])
                    # bias row rides as contraction row 34 (DMA can hit
                    # the unaligned partition offset)
                    nc.sync.dma_start(W_combT[IN:IN + 1, :], brow1[:])

            # ============ fp8 weight builds (all resident) ============
            with tc.tile_pool(name="wpool", bufs=1) as wpool:
                w1 = wpool.tile([P, KPF, 2, G], FP8, tag="Whh1")
                w2h = wpool.tile([P, KPF, 2, G], FP8, tag="Whh2")
                w2b = wpool.tile([P, KC, G], BF16, tag="Wih2b")
                with nc.named_scope("build_W8"):
                    with tc.tile_pool(name="wrow1", bufs=6) as wrow, \
                         tc.tile_pool(name="wtr_ps1", bufs=3, space="PSUM") as wtr_ps:
                        _build_weight_T8(nc, W_hh1, w1, identr, wrow, wtr_ps)
                        _build_weight_T8(nc, W_hh2, w2h, identr, wrow, wtr_ps)
                        # W_ih2^T in bf16 (x512) for the batched A2 GEMM
                        for r in range(G // P):
                            wt = wrow.tile([P, H], F32R, tag="wrow")
                            nc.sync.dma_start(
                                wt[:], W_ih2[r * P:(r + 1) * P, :].bitcast(F32R))
                            for kp in range(KPF):
                                pt = wtr_ps.tile([P, 2, P], F32R, tag="wtr")
                                for e in range(2):
                                    c = 2 * kp + e
                                    nc.tensor.transpose(
                                        pt[:, e, :], wt[:, c * P:(c + 1) * P],
                                        identr[:])
                                dst = w2b[:, 2 * kp:2 * kp + 2,
                                          r * P:(r + 1) * P]
                                if kp % 2 == 0:
                                    nc.vector.tensor_scalar_mul(dst, pt[:],
                                                                WSCALE * HSCALE)
                                else:
                                    nc.scalar.activation(dst, pt[:], AF.Copy,
                                                         scale=WSCALE * HSCALE)

                # persistent state
                hT8 = state.tile([P, KPF, 2, NSLOT, P], FP8, tag="hT8_ring")
                hTb = state.tile([P, KC, 2, 2, BPC], HDT, tag="hTb_ring")
                c_st = state.tile([P, H], F32, tag="c_stack")
                nc.gpsimd.memset(hT8[:].bitcast(mybir.dt.uint8), 0.0)
                nc.gpsimd.memset(hTb[:].bitcast(mybir.dt.uint16), 0.0)
                nc.gpsimd.memset(c_st[:], 0.0)

                # decode operands
                decWT_f = misc.tile([P, KC], F32, tag="decWT_f")
                nc.sync.dma_start(decWT_f[:], dec_W.rearrange("o (c p) -> p (c o)", p=P))
                decWT = misc.tile([P, KC], HDT, tag="decWT")
                nc.vector.tensor_copy(decWT[:], decWT_f[:])
                decb_f = misc.tile([1, 1], F32, tag="decb_f")
                nc.sync.dma_start(decb_f[:], dec_b[None, :])
                decb_sb = misc.tile([1, 1], HDT, tag="decb")
                nc.vector.tensor_copy(decb_sb[:], decb_f[:])
                ones_f = misc.tile([1, BPC], F32, tag="ones_f")
                nc.gpsimd.memset(ones_f[:], 1.0)
                ones_bpc = misc.tile([1, BPC], HDT, tag="ones_bpc")
                nc.vector.tensor_copy(ones_bpc[:], ones_f[:])
                hT_last = misc.tile([P, KC, BPC], HDT, tag="hT_last")

                # ============ fused recurrence loop ============
                with nc.named_scope("loop"):
                    with tc.tile_pool(name="l_g", bufs=4) as gact, \
                         tc.tile_pool(name="l_a2", bufs=2) as a2pool, \
                         tc.tile_pool(name="l_h", bufs=2) as hpool, \
                         tc.tile_pool(name="l_pg", bufs=3, space="PSUM") as psum_g, \
                         tc.tile_pool(name="l_ptr", bufs=2, space="PSUM") as psum_tr:
                        pg_next = {}
                        a2_cur = None
                        for t in range(T + 3):
                            do_l1 = t < T
                            do_l2 = t >= 3
                            r0 = 0 if do_l1 else BPC
                            r1 = P if do_l2 else BPC
                            s_r = (t - 1) % NSLOT
                            s_w = t % NSLOT
                            rh = BPC * ((t + 1) % 2)  # a2 row-half, L2-time t-3

                            def a2_block(chunks):
                                # batched bf16 A2 for L2-times (t-2, t-1),
                                # emitted in THIS step's tail where its h1^T
                                # inputs are >=1 step old (no PE wait) and the
                                # matmuls bridge the step-boundary bubble
                                pair = (t - 2) // 2
                                for chn in chunks:
                                    cs = slice(chn * 512, (chn + 1) * 512)
                                    pa = psum_tr.tile([P, 512], F32, tag="htr",
                                                      name="pa")
                                    for k in range(KC):
                                        nc.tensor.matmul(
                                            pa[:], hTb[:, k, pair % 2, :, :],
                                            w2b[:, k, cs],
                                            start=(k == 0), stop=(k == KC - 1),
                                            skip_group_check=True)
                                    nc.vector.tensor_add(a2_cur[:, cs], pa[:],
                                                         bias128_2[:, cs])

                            def mk_pgA(g_idx):
                                pgA = psum_g.tile([BPC, H], F32, tag="pg",
                                                  name=f"pgA{g_idx}")
                                for n2 in range(2):
                                    n = g_idx * 2 + n2
                                    nc.tensor.matmul(
                                        pgA[:, n2 * 512:(n2 + 1) * 512],
                                        xTa[:, t, :],
                                        W_combT[:, n * 512:(n + 1) * 512],
                                        start=True, stop=False,
                                        skip_group_check=True)
                                return pgA

                            def mm_l1(g_idx, pgA):
                                for n2 in range(2):
                                    n = g_idx * 2 + n2
                                    ch = slice(n2 * 512, (n2 + 1) * 512)
                                    wch = slice(n * 512, (n + 1) * 512)
                                    for kp in range(KPF - 1, -1, -1):
                                        nc.tensor.matmul(
                                            pgA[:, ch],
                                            hT8[:, kp, :, s_r, 0:BPC],
                                            w1[:, kp, :, wch],
                                            start=False, stop=(kp == 0),
                                            perf_mode=DR,
                                            skip_group_check=True)

                            def mm_l2(g_idx):
                                pgB = psum_g.tile([BPC, H], F32, tag="pg",
                                                  name=f"pgB{g_idx}")
                                # a2 (+bias) preset via DVE, then accumulate
                                nc.vector.tensor_copy(
                                    pgB[:],
                                    a2_cur[rh:rh + BPC,
                                           g_idx * H:(g_idx + 1) * H])
                                for n2 in range(2):
                                    n = g_idx * 2 + n2
                                    ch = slice(n2 * 512, (n2 + 1) * 512)
                                    wch = slice(n * 512, (n + 1) * 512)
                                    for kp in range(KPF - 1, -1, -1):
                                        nc.tensor.matmul(
                                            pgB[:, ch],
                                            hT8[:, kp, :, s_r, BPC:P],
                                            w2h[:, kp, :, wch],
                                            start=False, stop=(kp == 0),
                                            perf_mode=DR,
                                            skip_group_check=True)
                                return pgB

                            acts = {}

                            def do_gate(g_idx, func, name):
                                pgA = pg_next.pop(g_idx, None)
                                if do_l1:
                                    if pgA is None:
                                        pgA = mk_pgA(g_idx)
                                    mm_l1(g_idx, pgA)
                                pgB = mm_l2(g_idx) if do_l2 else None
                                at = gact.tile([P, H], HDT, tag="gact", name=name)
                                if do_l1:
                                    nc.scalar.activation(at[0:BPC], pgA[:], func,
                                                         bias=zb[0:BPC],
                                                         scale=INV_WSCALE)
                                if do_l2:
                                    nc.scalar.activation(at[BPC:P], pgB[:], func,
                                                         bias=zb[BPC:P],
                                                         scale=INV_WSCALE)
                                acts[g_idx] = at
                                return pgA, pgB

                            do_gate(0, AF.Sigmoid, "act_i")
                            do_gate(2, AF.Tanh, "act_g")
                            tmp = gact.tile([P, H], HDT, tag="gact", name="tmp")
                            nc.vector.tensor_mul(tmp[r0:r1], acts[0][r0:r1],
                                                 acts[2][r0:r1])

                            # gate f, then c update + tanh(c), in halves
                            pgA_f = pg_next.pop(1, None)
                            if do_l1:
                                if pgA_f is None:
                                    pgA_f = mk_pgA(1)
                                mm_l1(1, pgA_f)
                            pgB_f = mm_l2(1) if do_l2 else None
                            act_f = gact.tile([P, H], HDT, tag="gact", name="act_f")
                            tanh_c = gact.tile([P, H], HDT, tag="gact", name="tanh_c")
                            for hh in (1, 0):
                                sl = slice(hh * HF, (hh + 1) * HF)
                                if do_l1:
                                    nc.scalar.activation(act_f[0:BPC, sl],
                                                         pgA_f[:, sl], AF.Sigmoid,
                                                         bias=zb[0:BPC],
                                                         scale=INV_WSCALE)
                                if do_l2:
                                    nc.scalar.activation(act_f[BPC:P, sl],
                                                         pgB_f[:, sl], AF.Sigmoid,
                                                         bias=zb[BPC:P],
                                                         scale=INV_WSCALE)
                                nc.vector.tensor_mul(c_st[r0:r1, sl],
                                                     c_st[r0:r1, sl],
                                                     act_f[r0:r1, sl])
                                nc.vector.tensor_add(c_st[r0:r1, sl],
                                                     c_st[r0:r1, sl],
                                                     tmp[r0:r1, sl])
                                nc.scalar.activation(tanh_c[r0:r1, sl],
                                                     c_st[r0:r1, sl], AF.Tanh,
                                                     bias=zb[r0:r1])

                            # gate o + h, in halves
                            pgA_o = pg_next.pop(3, None)
                            if do_l1:
                                if pgA_o is None:
                                    pgA_o = mk_pgA(3)
                                mm_l1(3, pgA_o)
                            pgB_o = mm_l2(3) if do_l2 else None
                            act_o = gact.tile([P, H], HDT, tag="gact", name="act_o")
                            h_t = hpool.tile([P, H], HDT, tag="h_t")
                            for hh in (1, 0):
                                sl = slice(hh * HF, (hh + 1) * HF)
                                if do_l1:
                                    nc.scalar.activation(act_o[0:BPC, sl],
                                                         pgA_o[:, sl], AF.Sigmoid,
                                                         bias=zb[0:BPC],
                                                         scale=INV_WSCALE)
                                if do_l2:
                                    nc.scalar.activation(act_o[BPC:P, sl],
                                                         pgB_o[:, sl], AF.Sigmoid,
                                                         bias=zb[BPC:P],
                                                         scale=INV_WSCALE)
                                nc.vector.tensor_mul(h_t[r0:r1, sl],
                                                     act_o[r0:r1, sl],
                                                     tanh_c[r0:r1, sl])
                            # next-step L1 preloads (gates i, g, f) ride in
                            # the tail: independent PE work filling the bubble
                            if t + 1 < T:
                                for gi in (0, 2, 1):
                                    pgn = psum_g.tile([BPC, H], F32, tag="pg",
                                                      name=f"pgA{gi}")
                                    for n2 in range(2):
                                        n = gi * 2 + n2
                                        nc.tensor.matmul(
                                            pgn[:, n2 * 512:(n2 + 1) * 512],
                                            xTa[:, t + 1, :],
                                            W_combT[:, n * 512:(n + 1) * 512],
                                            start=True, stop=False,
                                            skip_group_check=True)
                                    pg_next[gi] = pgn

                            if 2 <= t <= T and t % 2 == 0:
                                a2_cur = a2pool.tile([P, G], HDT, tag="a2sb")
                                a2_block(range(0, 4))

                            # h^T transposes + ring writes (reversed: chunk
                            # 0, needed first next step, lands last)
                            if t <= T + 1:
                                for k in range(KC - 1, -1, -1):
                                    pt = psum_tr.tile([P, P], HDT, tag="htr")
                                    nc.tensor.transpose(
                                        pt[:, r0:r1],
                                        h_t[r0:r1, k * P:(k + 1) * P],
                                        identh[r0:r1, r0:r1])
                                    nc.vector.tensor_scalar_mul(
                                        hT8[:, k // 2, k % 2, s_w, r0:r1],
                                        pt[:, r0:r1], HSCALE)
                                    if do_l1:
                                        # bf16 h1^T pair-ring for the A2 GEMM
                                        nc.scalar.activation(
                                            hTb[:, k, (t // 2) % 2, t % 2, :],
                                            pt[:, 0:BPC], AF.Copy)
                                if 2 <= t <= T and t % 2 == 0:
                                    a2_block(range(4, 8))
                            else:
                                # final step: h2_{T-1}^T in bf16 for decode
                                for k in range(KC - 1, -1, -1):
                                    pt = psum_tr.tile([P, P], HDT, tag="htr")
                                    nc.tensor.transpose(
                                        pt[:, 0:BPC],
                                        h_t[BPC:P, k * P:(k + 1) * P],
                                        identh[BPC:P, BPC:P])
                                    nc.scalar.activation(
                                        hT_last[:, k, :], pt[:, 0:BPC], AF.Copy)

                        # decode: out = h2_{T-1} @ dec_W^T + dec_b
                        pd = psum_g.tile([1, BPC], F32, tag="pg", name="pdec")
                        for k in range(KC):
                            nc.tensor.matmul(pd[:], decWT[:, k:k + 1],
                                             hT_last[:, k, :],
                                             start=(k == 0), stop=False)
                        nc.tensor.matmul(pd[:], decb_sb[:], ones_bpc[:],
                                         start=False, stop=True)
                        osb = misc.tile([1, BPC], F32, tag="osb")
                        nc.vector.tensor_copy(osb[:], pd[:])
                        nc.sync.dma_start(out.rearrange("b o -> o b"), osb[:])

    nc.compile()
    return nc


_cached_nc = None
_cached_fn = None  # (jitted shard_map fn, in_names, out_names, out_shapes, zeros)


def _build_jitted(nc):
    """Same lowering as bass2jax.run_bass_via_pjrt, but the jitted
    executable is cached so repeat kernel() calls skip recompilation."""
    import jax
    from jax.sharding import Mesh, PartitionSpec
    from jax.experimental.shard_map import shard_map
    from concourse import bass2jax, mybir as _mybir

    bass2jax.install_neuronx_cc_hook()
    partition_name = nc.partition_id_tensor.name if nc.partition_id_tensor else None
    in_names, out_names, out_avals, zero_outs = [], [], [], []
    for alloc in nc.m.functions[0].allocations:
        if not isinstance(alloc, _mybir.MemoryLocationSet):
            continue
        name = alloc.memorylocations[0].name
        if alloc.kind == "ExternalInput":
            if name != partition_name:
                in_names.append(name)
        elif alloc.kind == "ExternalOutput":
            shape = tuple(alloc.tensor_shape)
            dtype = _mybir.dt.np(alloc.dtype)
            out_names.append(name)
            out_avals.append(jax.core.ShapedArray(shape, dtype))
            zero_outs.append(np.zeros(shape, dtype))
    n_params = len(in_names)
    n_outs = len(out_avals)
    all_in_names = list(in_names) + list(out_names)
    if partition_name is not None:
        all_in_names.append(partition_name)
    donate = tuple(range(n_params, n_params + n_outs))

    def _body(*args):
        operands = list(args)
        if partition_name is not None:
            operands.append(bass2jax.partition_id_tensor())
        outs = bass2jax._bass_exec_p.bind(
            *operands,
            out_avals=tuple(out_avals),
            in_names=tuple(all_in_names),
            out_names=tuple(out_names),
            lowering_input_output_aliases=(),
            sim_require_finite=True,
            sim_require_nnan=True,
            nc=nc,
        )
        return tuple(outs)

    devices = jax.devices()[:N_CORES]
    mesh = Mesh(np.asarray(devices), ("core",))
    in_specs = (PartitionSpec("core"),) * (n_params + n_outs)
    out_specs = (PartitionSpec("core"),) * n_outs
    fn = jax.jit(
        shard_map(_body, mesh=mesh, in_specs=in_specs, out_specs=out_specs,
                  check_rep=False),
        donate_argnums=donate, keep_unused=True,
    )
    out_shapes = [a.shape for a in out_avals]
    return fn, in_names, out_names, out_shapes, zero_outs


_dev_cache = {}  # name -> (digest, device_array)


def _to_device(name, arr):
    """Replicate-concat a weight to all cores and keep it on device across
    calls (keyed by content hash) so repeat kernel() calls only ship x."""
    import hashlib
    import jax
    d = hashlib.blake2b(arr.tobytes(), digest_size=16).digest()
    hit = _dev_cache.get(name)
    if hit is not None and hit[0] == d:
        return hit[1]
    conc = np.concatenate([arr] * N_CORES, axis=0)
    darr = jax.device_put(conc)
    _dev_cache[name] = (d, darr)
    return darr


def kernel(**inputs):
    global _cached_nc, _cached_fn
    if _cached_nc is None:
        _cached_nc = build(100)
        _cached_fn = _build_jitted(_cached_nc)
    fn, in_names, out_names, out_shapes, zero_outs = _cached_fn
    ins = {k: np.ascontiguousarray(np.asarray(v, dtype=np.float32))
           for k, v in inputs.items()}
    concat_in = []
    for name in in_names:
        if name == "x":
            concat_in.append(ins["x"])  # already [512, T, IN]; axis0 shards
        else:
            concat_in.append(_to_device(name, ins[name]))
    i = out_names.index("out")
    last_err = None
    for attempt in range(3):
        try:
            concat_zeros = [np.zeros((N_CORES * z.shape[0], *z.shape[1:]), z.dtype)
                            for z in zero_outs]
            out_arrs = fn(*concat_in, *concat_zeros)
            outp = np.asarray(out_arrs[i]).reshape(B, 1)
            return outp.astype(np.float32)
        except Exception as e:  # transient NRT_EXEC_UNIT_UNRECOVERABLE etc.
            last_err = e
            _dev_cache.clear()
            concat_in = []
            for name in in_names:
                if name == "x":
                    concat_in.append(ins["x"])
                else:
                    concat_in.append(_to_device(name, ins[name]))
    raise last_err


# revision 27
# speedup vs baseline: 1.1590x; 1.0710x over previous
"""Trainium2 Bass kernel for nn_Discriminator (2-layer LSTM, B=512 T=100 H=1024).

Strategy: data-parallel over batch across 8 cores (B=64 per core), with
both LSTM layers FUSED into one software-pipelined loop: macro-step t
computes layer-1 cell for time t and layer-2 cell for time t-2 (lag 2).
Gate preactivations accumulate in per-layer [64, 1024] PSUM tiles
(DoubleRow matmuls can only write PSUM partition 0), but the
activations write into ONE partition-stacked SBUF tile (L1 -> rows
0:64, L2 -> rows 64:128), so all downstream elementwise work (i*g, c
update, tanh, h) and the h^T transposes run once on [128, .] tiles for
both layers, and the PE always has >20us of mostly-independent matmul
work per step to hide the serial activation chain.

  - Recurrence products (h1@W_hh1^T, h2@W_hh2^T) are fp8e4m3 DoubleRow
    matmuls reading a shared fp8 h^T ring (cols 0:64 = h1, 64:128 =
    h2).  Weights are scaled x32 and h x16 before the fp8 cast
    (U(+-0.031) weights / small h are subnormal in e4m3 otherwise ->
    10-20% quantization error) and descaled for free via the
    activation `scale` operand.  Measured on HW: DR gives no
    per-instruction speedup (cost = N_out cycles regardless), but
    doubles K per instruction, which is what makes M=64 recurrences
    run at the same MAC rate as M=128 bf16 GEMMs.
  - The layer-2 input product h1@W_ih2^T MUST be bf16 on both operands
    (any fp8 operand pushes total rel err past the 2e-2 gate).  It is
    batched 2 timesteps at a time as an M=128 bf16 GEMM from a bf16
    h1^T ring (lag-2 makes both steps' h1 available), evicted
    PSUM->SBUF with the layer-2 bias added, then DVE-preset into the
    gate PSUM before the W_hh2 DoubleRow matmuls accumulate onto it
    (start=False onto DVE-written PSUM is legal).
  - Layer-1's input projection collapses through the encoder:
    W_comb = W_ih1 @ enc_W^T, and the per-step preload x_t @ W_comb^T
    rides a K=35 matmul whose lhsT carries [x_t ; ones]: the combined
    layer-1 bias lands with the projection.
  - Weight builds evict transposed k-pairs ([128, 2, 128]) in single
    DVE/Act ops -- evictions, not DMA, gate the prologue.
  - No DRAM scratch at all; HBM traffic is weights + x (~67MB/core,
    vs 199MB for the phase-separated baseline).  4.13ms -> 3.54ms.
"""

import numpy as np

import concourse.bass as bass
import concourse.tile as tile
import concourse.mybir as mybir
from concourse import bacc
from concourse.bass_utils import run_bass_kernel_spmd
from concourse.masks import make_identity

F32 = mybir.dt.float32
F32R = mybir.dt.float32r
BF16 = mybir.dt.bfloat16
FP8 = mybir.dt.float8e4
AF = mybir.ActivationFunctionType
DR = mybir.MatmulPerfMode.DoubleRow

N_CORES = 8
B, IN, H = 512, 34, 1024
G = 4 * H                 # 4096
BPC = B // N_CORES        # 64 batch rows per core
P = 128
KC = H // P               # 8 contraction chunks
KPF = KC // 2             # 4 fp8 k-pairs
NSLOT = 4                 # h^T ring depth
HDT = BF16
HF = 512                  # half of H for the split serial tail

WSCALE = 32.0             # fp8 weight pre-scale (exact power of 2)
HSCALE = 16.0             # fp8 h-ring pre-scale (fixes subnormal h)
INV_WSCALE = 1.0 / (WSCALE * HSCALE)


def _build_weight_T8(nc, w_dram, w_T8, identr, wrow, wtr_ps):
    """Transpose w_dram [G, H] into resident fp8 SBUF tile
    w_T8 [128, KPF, 2, G] (k-pair packed for DoubleRow), scaled by
    WSCALE in the PSUM->SBUF eviction (the PE transpose datapath
    ignores the identity operand's values, so scaling must not ride
    the transpose itself)."""
    n_row_tiles = w_dram.shape[0] // P  # 32
    for r in range(n_row_tiles):
        wt = wrow.tile([P, H], F32R, tag="wrow")
        nc.sync.dma_start(wt[:], w_dram[r * P:(r + 1) * P, :].bitcast(F32R))
        for kp in range(KPF):
            # two chunk transposes into one tile, ONE k-pair eviction
            # (evictions gate the build; halve their count)
            pt = wtr_ps.tile([P, 2, P], F32R, tag="wtr")
            for e in range(2):
                c = 2 * kp + e
                nc.tensor.transpose(pt[:, e, :], wt[:, c * P:(c + 1) * P],
                                    identr[:])
            dst = w_T8[:, kp, :, r * P:(r + 1) * P]
            # gpsimd cannot touch PSUM; alternate vector / scalar(Copy)
            if kp % 2 == 0:
                nc.vector.tensor_scalar_mul(dst, pt[:], WSCALE)
            else:
                nc.scalar.activation(dst, pt[:], AF.Copy, scale=WSCALE)


def build(T=100):
    nc = bacc.Bacc("TRN2", target_bir_lowering=False, debug=False,
                   num_devices=N_CORES)

    x = nc.dram_tensor("x", [BPC, T, IN], F32, kind="ExternalInput").ap()
    enc_W = nc.dram_tensor("enc_W", [H, IN], F32, kind="ExternalInput").ap()
    enc_b = nc.dram_tensor("enc_b", [H], F32, kind="ExternalInput").ap()
    W_ih1 = nc.dram_tensor("W_ih1", [G, H], F32, kind="ExternalInput").ap()
    W_hh1 = nc.dram_tensor("W_hh1", [G, H], F32, kind="ExternalInput").ap()
    b_ih1 = nc.dram_tensor("b_ih1", [G], F32, kind="ExternalInput").ap()
    b_hh1 = nc.dram_tensor("b_hh1", [G], F32, kind="ExternalInput").ap()
    W_ih2 = nc.dram_tensor("W_ih2", [G, H], F32, kind="ExternalInput").ap()
    W_hh2 = nc.dram_tensor("W_hh2", [G, H], F32, kind="ExternalInput").ap()
    b_ih2 = nc.dram_tensor("b_ih2", [G], F32, kind="ExternalInput").ap()
    b_hh2 = nc.dram_tensor("b_hh2", [G], F32, kind="ExternalInput").ap()
    dec_W = nc.dram_tensor("dec_W", [1, H], F32, kind="ExternalInput").ap()
    dec_b = nc.dram_tensor("dec_b", [1], F32, kind="ExternalInput").ap()
    out = nc.dram_tensor("out", [BPC, 1], F32, kind="ExternalOutput").ap()

    with tile.TileContext(nc) as tc:
        with tc.tile_pool(name="persist", bufs=1) as persist, \
             tc.tile_pool(name="state", bufs=1) as state, \
             tc.tile_pool(name="misc", bufs=1) as misc:

            ident = persist.tile([P, P], F32, tag="ident")
            make_identity(nc, ident[:])
            identr = persist.tile([P, P], F32R, tag="identr")
            nc.vector.tensor_copy(identr[:], ident[:])
            identh = persist.tile([P, P], HDT, tag="identh")
            nc.vector.tensor_copy(identh[:], ident[:])
            zb = persist.tile([P, 1], F32, tag="zero_bias")
            nc.gpsimd.memset(zb[:], 0.0)

            # layer-1 input-side operands, pre-scaled by WSCALE so the
            # PSUM accumulation matches the fp8 DR product scale:
            #   W_combT [35, G]: rows 0:34 = 32*(W_ih1@enc_W)^T,
            #     row 34 = 32*(enc_b@W_ih1^T + b_ih1 + b_hh1)
            #   xTa [35, T, 64]: per step t: [x_t^T ; ones]
            W_combT = persist.tile([IN + 1, G], BF16, tag="W_combT")
            xTa = persist.tile([IN + 1, T, BPC], BF16, tag="xTa")
            ones1 = persist.tile([1, P], F32R, tag="ones1")
            nc.gpsimd.memset(ones1[:].bitcast(F32), 1.0)
            # layer-2 bias broadcast [128, G], pre-scaled by WSCALE*HSCALE
            bias128_2 = persist.tile([P, G], BF16, tag="bias128_2")

            # ============ Phase E: xTa ============
            with nc.named_scope("phaseE"):
                with tc.tile_pool(name="e_sb", bufs=1) as e_sb, \
                     tc.tile_pool(name="e_ps", bufs=3, space="PSUM") as e_ps:
                    onesrow = e_sb.tile([1, T, BPC], BF16, tag="onesrow")
                    nc.gpsimd.memset(onesrow[:], 1.0)
                    nc.sync.dma_start(xTa[IN:IN + 1, :, :], onesrow[:])
                    # one bulk contiguous DMA of all of x (13.6KB/partition)
                    xall = e_sb.tile([BPC, T, IN], F32R, tag="xall")
                    nc.sync.dma_start(xall[:], x.bitcast(F32R))
                    for m in range(T // 2):
                        pt = e_ps.tile([IN, P], F32R, tag="xtr")
                        for e in range(2):
                            nc.tensor.transpose(
                                pt[:, e * BPC:(e + 1) * BPC],
                                xall[:, 2 * m + e, :], identr[0:BPC, 0:BPC])
                        nc.vector.tensor_copy(xTa[0:IN, 2 * m, :], pt[:, 0:BPC])
                        nc.scalar.activation(xTa[0:IN, 2 * m + 1, :], pt[:, BPC:P], AF.Copy)

            # ============ W_combT build (incremental, scaled) ============
            with nc.named_scope("build_Wcomb"):
                with tc.tile_pool(name="wc_sb", bufs=1) as wc_sb, \
                     tc.tile_pool(name="wc_row", bufs=6) as wc_row, \
                     tc.tile_pool(name="wc_st", bufs=2) as wc_st, \
                     tc.tile_pool(name="wc_ps", bufs=2, space="PSUM") as wc_ps, \
                     tc.tile_pool(name="wc_ps2", bufs=1, space="PSUM") as wc_ps2:
                    encwb = wc_sb.tile([P, KC, IN], F32R, tag="encwb")
                    nc.sync.dma_start(
                        encwb[:],
                        enc_W.rearrange("(c p) f -> p c f", p=P).bitcast(F32R))
                    encb_k = wc_sb.tile([P, KC], F32R, tag="encb_k")
                    nc.sync.dma_start(
                        encb_k[:],
                        enc_b.rearrange("(c p) -> p c", p=P).bitcast(F32R))
                    # WSCALE rides the encoder-side operands of the
                    # W_comb contraction (transposes don't scale)
                    nc.vector.tensor_scalar_mul(encwb[:], encwb[:], WSCALE * HSCALE)
                    nc.vector.tensor_scalar_mul(encb_k[:], encb_k[:], WSCALE * HSCALE)
                    brow1 = wc_sb.tile([1, G], BF16, tag="brow1")
                    bsum1 = wc_sb.tile([1, G], F32, tag="bsum1")
                    # pre-scaled bias sums; transient pool so the [1, G]
                    # f32 scratch frees before the weight staging runs
                    with tc.tile_pool(name="wc_tmp", bufs=1) as wc_tmp:
                        tA = wc_tmp.tile([1, G], F32, tag="tA")
                        nc.sync.dma_start(tA[:], b_ih1[None, :])
                        tB = wc_tmp.tile([1, G], F32, tag="tB")
                        nc.sync.dma_start(tB[:], b_hh1[None, :])
                        nc.vector.tensor_add(bsum1[:], tA[:], tB[:])
                        nc.gpsimd.tensor_scalar_mul(bsum1[:], bsum1[:], WSCALE * HSCALE)
                        tA = wc_tmp.tile([1, G], F32, tag="tA")
                        nc.sync.dma_start(tA[:], b_ih2[None, :])
                        tB = wc_tmp.tile([1, G], F32, tag="tB")
                        nc.sync.dma_start(tB[:], b_hh2[None, :])
                        nc.vector.tensor_add(tA[:], tA[:], tB[:])
                        nc.gpsimd.tensor_scalar_mul(tA[:], tA[:],
                                                    WSCALE * HSCALE)
                        brow2 = wc_tmp.tile([1, G], F32R, tag="brow2")
                        nc.vector.tensor_copy(brow2[:], tA[:])
                        for n in range(8):
                            slb = slice(n * 512, (n + 1) * 512)
                            pbb = wc_ps.tile([P, 512], F32, tag="pbb")
                            nc.tensor.matmul(pbb[:], ones1[:], brow2[:, slb],
                                             start=True, stop=True)
                            nc.vector.tensor_copy(bias128_2[:, slb], pbb[:])
                    # groups of 4 row-chunks = 512 G columns
                    for grp in range(G // 512):
                        wstage = wc_st.tile([P, KC, 512], F32R, tag="wstage")
                        for rr in range(4):
                            r = grp * 4 + rr
                            wt = wc_row.tile([P, H], F32R, tag="wcrow")
                            nc.sync.dma_start(
                                wt[:], W_ih1[r * P:(r + 1) * P, :].bitcast(F32R))
                            for kp in range(KPF):
                                ptr = wc_ps.tile([P, 2, P], F32R, tag="wctr")
                                for e in range(2):
                                    c = 2 * kp + e
                                    nc.tensor.transpose(
                                        ptr[:, e, :], wt[:, c * P:(c + 1) * P],
                                        identr[:])
                                dstw = wstage[:, 2 * kp:2 * kp + 2,
                                              rr * P:(rr + 1) * P]
                                if kp % 2 == 0:
                                    nc.vector.tensor_copy(dstw, ptr[:])
                                else:
                                    nc.scalar.activation(dstw, ptr[:], AF.Copy)
                        pb = wc_ps2.tile([IN, 512], F32, tag="wcpb")
                        pbias = wc_ps2.tile([1, 512], F32, tag="wcpbias")
                        for k in range(KC):
                            nc.tensor.matmul(pb[:], encwb[:, k, :],
                                             wstage[:, k, :],
                                             start=(k == 0), stop=(k == KC - 1))
                        for k in range(KC):
                            nc.tensor.matmul(pbias[:], encb_k[:, k:k + 1],
                                             wstage[:, k, :],
                                             start=(k == 0), stop=(k == KC - 1))
                        sl = slice(grp * 512, (grp + 1) * 512)
                        nc.vector.tensor_copy(W_combT[0:IN, sl], pb[:])
                        nc.vector.tensor_add(brow1[:, sl], pbias[:], bsum1[:, sl])
                    # bias row rides as contraction row 34 (DMA can hit
                    # the unaligned partition offset)
                    nc.sync.dma_start(W_combT[IN:IN + 1, :], brow1[:])

            # ============ fp8 weight builds (all resident) ============
            with tc.tile_pool(name="wpool", bufs=1) as wpool:
                w1 = wpool.tile([P, KPF, 2, G], FP8, tag="Whh1")
                w2h = wpool.tile([P, KPF, 2, G], FP8, tag="Whh2")
                w2b = wpool.tile([P, KC, G], BF16, tag="Wih2b")
                with nc.named_scope("build_W8"):
                    with tc.tile_pool(name="wrow1", bufs=6) as wrow, \
                         tc.tile_pool(name="wtr_ps1", bufs=3, space="PSUM") as wtr_ps:
                        _build_weight_T8(nc, W_hh1, w1, identr, wrow, wtr_ps)
                        _build_weight_T8(nc, W_hh2, w2h, identr, wrow, wtr_ps)
                        # W_ih2^T in bf16 (x512) for the batched A2 GEMM
                        for r in range(G // P):
                            wt = wrow.tile([P, H], F32R, tag="wrow")
                            nc.sync.dma_start(
                                wt[:], W_ih2[r * P:(r + 1) * P, :].bitcast(F32R))
                            for kp in range(KPF):
                                pt = wtr_ps.tile([P, 2, P], F32R, tag="wtr")
                                for e in range(2):
                                    c = 2 * kp + e
                                    nc.tensor.transpose(
                                        pt[:, e, :], wt[:, c * P:(c + 1) * P],
                                        identr[:])
                                dst = w2b[:, 2 * kp:2 * kp + 2,
                                          r * P:(r + 1) * P]
                                if kp % 2 == 0:
                                    nc.vector.tensor_scalar_mul(dst, pt[:],
                                                                WSCALE * HSCALE)
                                else:
                                    nc.scalar.activation(dst, pt[:], AF.Copy,
                                                         scale=WSCALE * HSCALE)

                # persistent state
                hT8 = state.tile([P, KPF, 2, NSLOT, P], FP8, tag="hT8_ring")
                hTb = state.tile([P, KC, 2, 2, BPC], HDT, tag="hTb_ring")
                c_st = state.tile([P, H], F32, tag="c_stack")
                nc.gpsimd.memset(hT8[:].bitcast(mybir.dt.uint8), 0.0)
                nc.gpsimd.memset(hTb[:].bitcast(mybir.dt.uint16), 0.0)
                nc.gpsimd.memset(c_st[:], 0.0)

                # decode operands
                decWT_f = misc.tile([P, KC], F32, tag="decWT_f")
                nc.sync.dma_start(decWT_f[:], dec_W.rearrange("o (c p) -> p (c o)", p=P))
                decWT = misc.tile([P, KC], HDT, tag="decWT")
                nc.vector.tensor_copy(decWT[:], decWT_f[:])
                decb_f = misc.tile([1, 1], F32, tag="decb_f")
                nc.sync.dma_start(decb_f[:], dec_b[None, :])
                decb_sb = misc.tile([1, 1], HDT, tag="decb")
                nc.vector.tensor_copy(decb_sb[:], decb_f[:])
                ones_f = misc.tile([1, BPC], F32, tag="ones_f")
                nc.gpsimd.memset(ones_f[:], 1.0)
                ones_bpc = misc.tile([1, BPC], HDT, tag="ones_bpc")
                nc.vector.tensor_copy(ones_bpc[:], ones_f[:])
                hT_last = misc.tile([P, KC, BPC], HDT, tag="hT_last")

                # ============ fused recurrence loop ============
                with nc.named_scope("loop"):
                    with tc.tile_pool(name="l_g", bufs=4) as gact, \
                         tc.tile_pool(name="l_a2", bufs=2) as a2pool, \
                         tc.tile_pool(name="l_h", bufs=2) as hpool, \
                         tc.tile_pool(name="l_pg", bufs=3, space="PSUM") as psum_g, \
                         tc.tile_pool(name="l_ptr", bufs=2, space="PSUM") as psum_tr:
                        pg_next = {}
                        a2_cur = None
                        for t in range(T + 3):
                            do_l1 = t < T
                            do_l2 = t >= 3
                            r0 = 0 if do_l1 else BPC
                            r1 = P if do_l2 else BPC
                            s_r = (t - 1) % NSLOT
                            s_w = t % NSLOT
                            rh = BPC * ((t + 1) % 2)  # a2 row-half, L2-time t-3

                            def a2_block(chunks):
                                # batched bf16 A2 for L2-times (t-2, t-1),
                                # emitted in THIS step's tail where its h1^T
                                # inputs are >=1 step old (no PE wait) and the
                                # matmuls bridge the step-boundary bubble
                                pair = (t - 2) // 2
                                for chn in chunks:
                                    cs = slice(chn * 512, (chn + 1) * 512)
                                    pa = psum_tr.tile([P, 512], F32, tag="htr",
                                                      name="pa")
                                    for k in range(KC):
                                        nc.tensor.matmul(
                                            pa[:], hTb[:, k, pair % 2, :, :],
                                            w2b[:, k, cs],
                                            start=(k == 0), stop=(k == KC - 1),
                                            skip_group_check=True)
                                    nc.vector.tensor_add(a2_cur[:, cs], pa[:],
                                                         bias128_2[:, cs])

                            def mk_pgA(g_idx):
                                pgA = psum_g.tile([BPC, H], F32, tag="pg",
                                                  name=f"pgA{g_idx}")
                                for n2 in range(2):
                                    n = g_idx * 2 + n2
                                    nc.tensor.matmul(
                                        pgA[:, n2 * 512:(n2 + 1) * 512],
                                        xTa[:, t, :],
                                        W_combT[:, n * 512:(n + 1) * 512],
                                        start=True, stop=False,
                                        skip_group_check=True)
                                return pgA

                            def mm_l1(g_idx, pgA):
                                for n2 in range(2):
                                    n = g_idx * 2 + n2
                                    ch = slice(n2 * 512, (n2 + 1) * 512)
                                    wch = slice(n * 512, (n + 1) * 512)
                                    for kp in range(KPF - 1, -1, -1):
                                        nc.tensor.matmul(
                                            pgA[:, ch],
                                            hT8[:, kp, :, s_r, 0:BPC],
                                            w1[:, kp, :, wch],
                                            start=False, stop=(kp == 0),
                                            perf_mode=DR,
                                            skip_group_check=True)

                            def mm_l2(g_idx):
                                pgB = psum_g.tile([BPC, H], F32, tag="pg",
                                                  name=f"pgB{g_idx}")
                                # a2 (+bias) preset via DVE, then accumulate
                                nc.vector.tensor_copy(
                                    pgB[:],
                                    a2_cur[rh:rh + BPC,
                                           g_idx * H:(g_idx + 1) * H])
                                for n2 in range(2):
                                    n = g_idx * 2 + n2
                                    ch = slice(n2 * 512, (n2 + 1) * 512)
                                    wch = slice(n * 512, (n + 1) * 512)
                                    for kp in range(KPF - 1, -1, -1):
                                        nc.tensor.matmul(
                                            pgB[:, ch],
                                            hT8[:, kp, :, s_r, BPC:P],
                                            w2h[:, kp, :, wch],
                                            start=False, stop=(kp == 0),
                                            perf_mode=DR,
                                            skip_group_check=True)
                                return pgB

                            acts = {}

                            def do_gate(g_idx, func, name):
                                pgA = pg_next.pop(g_idx, None)
                                if do_l1:
                                    if pgA is None:
                                        pgA = mk_pgA(g_idx)
                                    mm_l1(g_idx, pgA)
                                pgB = mm_l2(g_idx) if do_l2 else None
                                at = gact.tile([P, H], HDT, tag="gact", name=name)
                                if do_l1:
                                    nc.scalar.activation(at[0:BPC], pgA[:], func,
                                                         bias=zb[0:BPC],
                                                         scale=INV_WSCALE)
                                if do_l2:
                                    nc.scalar.activation(at[BPC:P], pgB[:], func,
                                                         bias=zb[BPC:P],
                                                         scale=INV_WSCALE)
                                acts[g_idx] = at
                                return pgA, pgB

                            do_gate(0, AF.Sigmoid, "act_i")
                            do_gate(2, AF.Tanh, "act_g")
                            tmp = gact.tile([P, H], HDT, tag="gact", name="tmp")
                            nc.vector.tensor_mul(tmp[r0:r1], acts[0][r0:r1],
                                                 acts[2][r0:r1])

                            # gate f, then c update + tanh(c), in halves
                            pgA_f = pg_next.pop(1, None)
                            if do_l1:
                                if pgA_f is None:
                                    pgA_f = mk_pgA(1)
                                mm_l1(1, pgA_f)
                            pgB_f = mm_l2(1) if do_l2 else None
                            act_f = gact.tile([P, H], HDT, tag="gact", name="act_f")
                            tanh_c = gact.tile([P, H], HDT, tag="gact", name="tanh_c")
                            for hh in (1, 0):
                                sl = slice(hh * HF, (hh + 1) * HF)
                                if do_l1:
                                    nc.scalar.activation(act_f[0:BPC, sl],
                                                         pgA_f[:, sl], AF.Sigmoid,
                                                         bias=zb[0:BPC],
                                                         scale=INV_WSCALE)
                                if do_l2:
                                    nc.scalar.activation(act_f[BPC:P, sl],
                                                         pgB_f[:, sl], AF.Sigmoid,
                                                         bias=zb[BPC:P],
                                                         scale=INV_WSCALE)
                                nc.vector.tensor_mul(c_st[r0:r1, sl],
                                                     c_st[r0:r1, sl],
                                                     act_f[r0:r1, sl])
                                nc.vector.tensor_add(c_st[r0:r1, sl],
                                                     c_st[r0:r1, sl],
                                                     tmp[r0:r1, sl])
                                nc.scalar.activation(tanh_c[r0:r1, sl],
                                                     c_st[r0:r1, sl], AF.Tanh,
                                                     bias=zb[r0:r1])

                            # gate o + h, in halves
                            pgA_o = pg_next.pop(3, None)
                            if do_l1:
                                if pgA_o is None:
                                    pgA_o = mk_pgA(3)
                                mm_l1(3, pgA_o)
                            pgB_o = mm_l2(3) if do_l2 else None
                            act_o = gact.tile([P, H], HDT, tag="gact", name="act_o")
                            h_t = hpool.tile([P, H], HDT, tag="h_t")
                            for hh in (1, 0):
                                sl = slice(hh * HF, (hh + 1) * HF)
                                if do_l1:
                                    nc.scalar.activation(act_o[0:BPC, sl],
                                                         pgA_o[:, sl], AF.Sigmoid,
                                                         bias=zb[0:BPC],
                                                         scale=INV_WSCALE)
                                if do_l2:
                                    nc.scalar.activation(act_o[BPC:P, sl],
                                                         pgB_o[:, sl], AF.Sigmoid,
                                                         bias=zb[BPC:P],
                                                         scale=INV_WSCALE)
                                nc.vector.tensor_mul(h_t[r0:r1, sl],
                                                     act_o[r0:r1, sl],
                                                     tanh_c[r0:r1, sl])
                            # next-step L1 preloads (gates i, g, f) ride in
                            # the tail: independent PE work filling the bubble
                            if t + 1 < T:
                                for gi in (0, 2, 1):
                                    pgn = psum_g.tile([BPC, H], F32, tag="pg",
                                                      name=f"pgA{gi}")
                                    for n2 in range(2):
                                        n = gi * 2 + n2
                                        nc.tensor.matmul(
                                            pgn[:, n2 * 512:(n2 + 1) * 512],
                                            xTa[:, t + 1, :],
                                            W_combT[:, n * 512:(n + 1) * 512],
                                            start=True, stop=False,
                                            skip_group_check=True)
                                    pg_next[gi] = pgn

                            if 2 <= t <= T and t % 2 == 0:
                                a2_cur = a2pool.tile([P, G], HDT, tag="a2sb")
                                a2_block(range(0, 4))

                            # h^T transposes + ring writes (reversed: chunk
                            # 0, needed first next step, lands last)
                            if t <= T + 1:
                                for kp in range(KPF - 1, -1, -1):
                                    # paired transposes -> single k-pair ring
                                    # copies (halves tail DVE/Act op count)
                                    pt = psum_tr.tile([P, 2, P], HDT, tag="htr")
                                    for e in (1, 0):
                                        k = 2 * kp + e
                                        nc.tensor.transpose(
                                            pt[:, e, r0:r1],
                                            h_t[r0:r1, k * P:(k + 1) * P],
                                            identh[r0:r1, r0:r1])
                                    nc.vector.tensor_scalar_mul(
                                        hT8[:, kp, :, s_w, r0:r1],
                                        pt[:, :, r0:r1], HSCALE)
                                    if do_l1:
                                        # bf16 h1^T pair-ring for the A2 GEMM
                                        nc.scalar.activation(
                                            hTb[:, 2 * kp:2 * kp + 2,
                                                (t // 2) % 2, t % 2, :],
                                            pt[:, :, 0:BPC], AF.Copy)
                                if 2 <= t <= T and t % 2 == 0:
                                    a2_block(range(4, 8))
                            else:
                                # final step: h2_{T-1}^T in bf16 for decode
                                for k in range(KC - 1, -1, -1):
                                    pt = psum_tr.tile([P, P], HDT, tag="htr")
                                    nc.tensor.transpose(
                                        pt[:, 0:BPC],
                                        h_t[BPC:P, k * P:(k + 1) * P],
                                        identh[BPC:P, BPC:P])
                                    nc.scalar.activation(
                                        hT_last[:, k, :], pt[:, 0:BPC], AF.Copy)

                        # decode: out = h2_{T-1} @ dec_W^T + dec_b
                        pd = psum_g.tile([1, BPC], F32, tag="pg", name="pdec")
                        for k in range(KC):
                            nc.tensor.matmul(pd[:], decWT[:, k:k + 1],
                                             hT_last[:, k, :],
                                             start=(k == 0), stop=False)
                        nc.tensor.matmul(pd[:], decb_sb[:], ones_bpc[:],
                                         start=False, stop=True)
                        osb = misc.tile([1, BPC], F32, tag="osb")
                        nc.vector.tensor_copy(osb[:], pd[:])
                        nc.sync.dma_start(out.rearrange("b o -> o b"), osb[:])

    nc.compile()
    return nc


_cached_nc = None
_cached_fn = None  # (jitted shard_map fn, in_names, out_names, out_shapes, zeros)


def _build_jitted(nc):
    """Same lowering as bass2jax.run_bass_via_pjrt, but the jitted
    executable is cached so repeat kernel() calls skip recompilation."""
    import jax
    from jax.sharding import Mesh, PartitionSpec
    from jax.experimental.shard_map import shard_map
    from concourse import bass2jax, mybir as _mybir

    bass2jax.install_neuronx_cc_hook()
    partition_name = nc.partition_id_tensor.name if nc.partition_id_tensor else None
    in_names, out_names, out_avals, zero_outs = [], [], [], []
    for alloc in nc.m.functions[0].allocations:
        if not isinstance(alloc, _mybir.MemoryLocationSet):
            continue
        name = alloc.memorylocations[0].name
        if alloc.kind == "ExternalInput":
            if name != partition_name:
                in_names.append(name)
        elif alloc.kind == "ExternalOutput":
            shape = tuple(alloc.tensor_shape)
            dtype = _mybir.dt.np(alloc.dtype)
            out_names.append(name)
            out_avals.append(jax.core.ShapedArray(shape, dtype))
            zero_outs.append(np.zeros(shape, dtype))
    n_params = len(in_names)
    n_outs = len(out_avals)
    all_in_names = list(in_names) + list(out_names)
    if partition_name is not None:
        all_in_names.append(partition_name)
    donate = tuple(range(n_params, n_params + n_outs))

    def _body(*args):
        operands = list(args)
        if partition_name is not None:
            operands.append(bass2jax.partition_id_tensor())
        outs = bass2jax._bass_exec_p.bind(
            *operands,
            out_avals=tuple(out_avals),
            in_names=tuple(all_in_names),
            out_names=tuple(out_names),
            lowering_input_output_aliases=(),
            sim_require_finite=True,
            sim_require_nnan=True,
            nc=nc,
        )
        return tuple(outs)

    devices = jax.devices()[:N_CORES]
    mesh = Mesh(np.asarray(devices), ("core",))
    in_specs = (PartitionSpec("core"),) * (n_params + n_outs)
    out_specs = (PartitionSpec("core"),) * n_outs
    fn = jax.jit(
        shard_map(_body, mesh=mesh, in_specs=in_specs, out_specs=out_specs,
                  check_rep=False),
        donate_argnums=donate, keep_unused=True,
    )
    out_shapes = [a.shape for a in out_avals]
    return fn, in_names, out_names, out_shapes, zero_outs


_dev_cache = {}  # name -> (digest, device_array)


def _to_device(name, arr):
    """Replicate-concat a weight to all cores and keep it on device across
    calls (keyed by content hash) so repeat kernel() calls only ship x."""
    import hashlib
    import jax
    d = hashlib.blake2b(arr.tobytes(), digest_size=16).digest()
    hit = _dev_cache.get(name)
    if hit is not None and hit[0] == d:
        return hit[1]
    conc = np.concatenate([arr] * N_CORES, axis=0)
    darr = jax.device_put(conc)
    _dev_cache[name] = (d, darr)
    return darr


def kernel(**inputs):
    global _cached_nc, _cached_fn
    if _cached_nc is None:
        _cached_nc = build(100)
        _cached_fn = _build_jitted(_cached_nc)
    fn, in_names, out_names, out_shapes, zero_outs = _cached_fn
    ins = {k: np.ascontiguousarray(np.asarray(v, dtype=np.float32))
           for k, v in inputs.items()}
    concat_in = []
    for name in in_names:
        if name == "x":
            concat_in.append(ins["x"])  # already [512, T, IN]; axis0 shards
        else:
            concat_in.append(_to_device(name, ins[name]))
    i = out_names.index("out")
    last_err = None
    for attempt in range(3):
        try:
            concat_zeros = [np.zeros((N_CORES * z.shape[0], *z.shape[1:]), z.dtype)
                            for z in zero_outs]
            out_arrs = fn(*concat_in, *concat_zeros)
            outp = np.asarray(out_arrs[i]).reshape(B, 1)
            return outp.astype(np.float32)
        except Exception as e:  # transient NRT_EXEC_UNIT_UNRECOVERABLE etc.
            last_err = e
            _dev_cache.clear()
            concat_in = []
            for name in in_names:
                if name == "x":
                    concat_in.append(ins["x"])
                else:
                    concat_in.append(_to_device(name, ins[name]))
    raise last_err
